# revision 1
# baseline (speedup 1.0000x reference)
"""Trainium2 Bass kernel for nn_GroundPropagation.

Structure (8 NeuronCores, batch-parallel, one batch element per core):

Phase 1 (device): per-channel reductions of s = sigmoid(x):
  - row sums  (C, H)  : sum over W of s           -> host computes disp/depth dots
  - sum of s^2 (C,)   : for the channel norms
Host: combines per-core partials in f64, ranks channels by cosine
  similarity against the disparity/depth ramps, picks top-16 + top-16.

Phase 2 (device): for the 32 selected channels, the 32-iteration masked
  "pull value from the row below" propagation collapses algebraically to
  a single bottom-up first-order recurrence per column:
      state = m[t] * state + (1 - m[t]) * sel[t]
  which is exactly one `tensor_tensor_scan` instruction per core
  (columns are packed per-partition; the mask is forced to 0 at each
  column's bottom row so the state resets at column boundaries).
  This is exact whenever no column has >= 33 consecutive masked rows
  (checked on host; P ~ 2^-33 per site otherwise).
  Then the clip-normalized blend weight and the final mix are computed
  and written back; host scatters the 32 channels into the full tensor.
"""

import sys

sys.path.insert(0, "/opt/trn_rl_repo")

import numpy as np

B, C, H, W = 8, 128, 96, 320
HW = H * W                  # 30720
NSEL = 16
NS = 2 * NSEL               # 32 selected channels
CLIP = 0.3
EPS = 1e-6
N_CORES = 8

NCH1, CH1 = 4, HW // 4      # phase-1 chunks (128, 7680)
WQ = 4                      # w-quarters; partition p = wq*32 + ch
WPQ = W // WQ               # 80 columns per quarter
S2 = WPQ * H                # 7680 free elems per partition in phase 2
NCH2 = 4
CH2 = S2 // NCH2            # 1920 = 20 columns of 96
NSQ = 2                     # phase-1 chunks whose s^2 runs on ACT (rest on DVE)

_cache = {}


def _runner(nc, n_cores):
    """Build a cached jitted callable for this Bass program via PJRT
    (mirrors concourse.bass2jax.run_bass_via_pjrt, but reusable)."""
    import jax
    from concourse import mybir
    from concourse.bass2jax import (
        _bass_exec_p,
        install_neuronx_cc_hook,
        partition_id_tensor,
    )
    from jax.sharding import Mesh, PartitionSpec
    from jax.experimental.shard_map import shard_map

    install_neuronx_cc_hook()
    partition_name = nc.partition_id_tensor.name if nc.partition_id_tensor else None

    in_names, out_names, out_avals = [], [], []
    for alloc in nc.m.functions[0].allocations:
        if not isinstance(alloc, mybir.MemoryLocationSet):
            continue
        name = alloc.memorylocations[0].name
        if alloc.kind == "ExternalInput":
            if name != partition_name:
                in_names.append(name)
        elif alloc.kind == "ExternalOutput":
            out_names.append(name)
            out_avals.append(
                jax.core.ShapedArray(
                    tuple(alloc.tensor_shape), mybir.dt.np(alloc.dtype)
                )
            )
    n_params = len(in_names)
    n_outs = len(out_avals)
    all_names = in_names + out_names + ([partition_name] if partition_name else [])
    donate = tuple(range(n_params, n_params + n_outs))

    def _body(*args):
        operands = list(args)
        if partition_name is not None:
            operands.append(partition_id_tensor())
        outs = _bass_exec_p.bind(
            *operands,
            out_avals=tuple(out_avals),
            in_names=tuple(all_names),
            out_names=tuple(out_names),
            lowering_input_output_aliases=(),
            sim_require_finite=True,
            sim_require_nnan=True,
            nc=nc,
        )
        return tuple(outs)

    devices = jax.devices()[:n_cores]
    mesh = Mesh(np.asarray(devices), ("core",))
    in_specs = (PartitionSpec("core"),) * (n_params + n_outs)
    out_specs = (PartitionSpec("core"),) * n_outs
    sharded = jax.jit(
        shard_map(
            _body, mesh=mesh, in_specs=in_specs, out_specs=out_specs, check_rep=False
        ),
        donate_argnums=donate,
        keep_unused=True,
    )

    def run(in_maps):
        concat_in = [
            np.concatenate([np.asarray(m[name]) for m in in_maps], axis=0)
            for name in in_names
        ]
        zeros = [
            np.zeros((n_cores * a.shape[0], *a.shape[1:]), a.dtype) for a in out_avals
        ]
        out_arrs = sharded(*concat_in, *zeros)
        return [
            {
                name: np.asarray(out_arrs[i]).reshape(
                    n_cores, *out_avals[i].shape
                )[c]
                for i, name in enumerate(out_names)
            }
            for c in range(n_cores)
        ]

    return run


def build_phase1():
    from contextlib import ExitStack

    import concourse.tile as tile
    from concourse import bacc, mybir

    f32 = mybir.dt.float32
    nc = bacc.Bacc("TRN2", target_bir_lowering=False, debug=False,
                   num_devices=N_CORES)
    x = nc.dram_tensor("x", (C, HW), f32, kind="ExternalInput").ap()
    rows = nc.dram_tensor("rows", (C, H), f32, kind="ExternalOutput").ap()
    ssq = nc.dram_tensor("ssq", (C, NCH1), f32, kind="ExternalOutput").ap()
    HC = H // NCH1  # rows per chunk

    with tile.TileContext(nc) as tc, ExitStack() as ctx:
        px = ctx.enter_context(tc.tile_pool(name="px", bufs=2))
        ps = ctx.enter_context(tc.tile_pool(name="ps", bufs=2))
        psq = ctx.enter_context(tc.tile_pool(name="psq", bufs=2))
        psm = ctx.enter_context(tc.tile_pool(name="psm", bufs=1))

        rows_sb = psm.tile([C, H], f32)
        ssq_a = psm.tile([C, NSQ], f32)
        ssq_d = psm.tile([C, NCH1 - NSQ], f32)
        for i in range(NCH1):
            xt = px.tile([C, CH1], f32, tag="x")
            nc.sync.dma_start(xt[:], x[:, i * CH1:(i + 1) * CH1])
            st = ps.tile([C, CH1], f32, tag="s")
            nc.scalar.activation(st[:], xt[:], mybir.ActivationFunctionType.Sigmoid)
            nc.vector.tensor_reduce(
                rows_sb[:, i * HC:(i + 1) * HC],
                st[:].rearrange("p (h w) -> p h w", w=W),
                mybir.AxisListType.X,
                mybir.AluOpType.add,
            )
            sq = psq.tile([C, CH1], f32, tag="sq")
            if i < NSQ:
                nc.scalar.activation(
                    sq[:], st[:], mybir.ActivationFunctionType.Square,
                    accum_out=ssq_a[:, i:i + 1],
                )
            else:
                nc.vector.scalar_tensor_tensor(
                    sq[:], st[:], 1.0, st[:],
                    op0=mybir.AluOpType.mult, op1=mybir.AluOpType.mult,
                    accum_out=ssq_d[:, i - NSQ:i - NSQ + 1],
                )
        nc.sync.dma_start(rows[:], rows_sb[:])
        nc.sync.dma_start(ssq[:, :NSQ], ssq_a[:])
        nc.sync.dma_start(ssq[:, NSQ:], ssq_d[:])
    nc.compile()
    return nc


def build_phase2():
    from contextlib import ExitStack

    import concourse.tile as tile
    from concourse import bacc, mybir

    f32 = mybir.dt.float32
    u8 = mybir.dt.uint8
    Alu = mybir.AluOpType
    Act = mybir.ActivationFunctionType
    nc = bacc.Bacc("TRN2", target_bir_lowering=False, debug=False,
                   num_devices=N_CORES)
    sel = nc.dram_tensor("sel", (C, S2), f32, kind="ExternalInput").ap()
    msk = nc.dram_tensor("msk", (C, S2), u8, kind="ExternalInput").ap()
    ref = nc.dram_tensor("ref", (C, S2), f32, kind="ExternalOutput").ap()
    NB = CH2 // 32  # 32-col blocks per chunk (60)

    with tile.TileContext(nc) as tc, ExitStack() as ctx:
        pools = {}
        for name, bufs in [("sel", NCH2 + 1), ("m", 3), ("qa", 3),
                           ("vw", 3), ("d", NCH2 + 1), ("tb", 3),
                           ("wb", 3), ("wr", 3), ("rf", 3), ("sm", 1)]:
            pools[name] = ctx.enter_context(tc.tile_pool(name=name, bufs=bufs))
        for name in ("ps1", "ps2"):
            pools[name] = ctx.enter_context(
                tc.tile_pool(name=name, bufs=2, space="PSUM"))
        from concourse.masks import make_identity
        ident = pools["sm"].tile([C, C], f32)
        make_identity(nc, ident[:])

        psm = pools["sm"]
        mxp = psm.tile([C, NCH2], f32)
        mxr = psm.tile([C, 1], f32)
        mrow = psm.tile([1, C], f32)
        Mc = psm.tile([1, NS], f32)
        zc = psm.tile([1, NS], f32)
        den = psm.tile([1, NS], f32)
        rc1 = psm.tile([1, NS], f32)
        rc4 = psm.tile([1, C], f32)
        rcp = psm.tile([C, 1], f32)
        wred = psm.tile([C, NCH2 * NB], f32)

        selts, mts, dts, ats = [], [], [], []
        # --- stage I: load, q, scan, d, |d|, per-chunk max ---
        for i in range(NCH2):
            sl = slice(i * CH2, (i + 1) * CH2)
            selt = pools["sel"].tile([C, CH2], f32, tag="sel")
            nc.sync.dma_start(selt[:], sel[:, sl])
            mt = pools["m"].tile([C, CH2], u8, tag="m")
            nc.sync.dma_start(mt[:], msk[:, sl])
            qt = pools["qa"].tile([C, CH2], f32, tag="qa")
            # q = (m == 0) * sel
            nc.vector.scalar_tensor_tensor(
                qt[:], mt[:], 0.0, selt[:], op0=Alu.is_equal, op1=Alu.mult)
            Vt = pools["vw"].tile([C, CH2], f32, tag="vw")
            # state = m*state + q   (bottom-up propagation, per column)
            nc.vector.tensor_tensor_scan(
                Vt[:], mt[:], qt[:], 0.0, op0=Alu.mult, op1=Alu.add)
            dt = pools["d"].tile([C, CH2], f32, tag="d")
            nc.gpsimd.tensor_tensor(dt[:], Vt[:], selt[:], Alu.subtract)
            nc.vector.tensor_reduce(
                mxp[:, i:i + 1], dt[:], mybir.AxisListType.X, Alu.max,
                apply_absolute_value=True)
            selts.append(selt); mts.append(mt); dts.append(dt)

        # --- barrier: per-(b,c) max over space -> 1/m_clip per channel ---
        nc.vector.tensor_reduce(mxr[:], mxp[:], mybir.AxisListType.X, Alu.max)
        nc.sync.dma_start(mrow[:], mxr[:])  # (128,1) -> (1,128)
        nc.vector.tensor_reduce(
            Mc[:], mrow[:].rearrange("o (q c) -> o c q", q=WQ),
            mybir.AxisListType.X, Alu.max)
        nc.vector.tensor_scalar(zc[:], Mc[:], 0.0, None, op0=Alu.is_equal)
        nc.vector.scalar_tensor_tensor(
            den[:], Mc[:], CLIP, zc[:], op0=Alu.mult, op1=Alu.add)
        nc.vector.reciprocal(rc1[:], den[:])
        # broadcast (1,32) -> (1,128) on DVE, then DMA to per-partition (128,1)
        nc.vector.tensor_copy(
            rc4[:].rearrange("o (q c) -> o q c", q=WQ),
            rc1[:].unsqueeze(1).broadcast_to((1, WQ, NS)))
        nc.sync.dma_start(rcp[:], rc4[:])

        # --- stage II: w_px on ACT, channel max via PE transposes, blend ---
        SPLITS = [(0, 1024), (1024, 896)]  # 128-aligned sub-chunks per chunk
        for i in range(NCH2):
            for off, ln in SPLITS:
                nt = ln // 128
                sl = slice(i * CH2 + off, i * CH2 + off + ln)
                dsl = slice(off, off + ln)
                # w_px = |d| / m_clip on ACT (clip to 1 after the channel max)
                wpx = pools["vw"].tile([C, ln], f32, tag="vw",
                                       padded_shape=[C, 1024])
                nc.scalar.activation(wpx[:], dts[i][:, dsl], Act.Abs,
                                     scale=rcp[:])
                # transpose to (pos, (wq, ch)) on PE
                t1p = pools["ps1"].tile([C, ln], f32, tag="ps1", space="PSUM",
                                        padded_shape=[C, 1024])
                for t in range(nt):
                    ts = slice(t * 128, (t + 1) * 128)
                    nc.tensor.transpose(t1p[:, ts], wpx[:, ts], ident[:])
                # max over ch within each (tile, wq); then clip at 1
                wrd = pools["wr"].tile([C, nt * WQ], f32, tag="wr",
                                       padded_shape=[C, 32])
                nc.vector.tensor_reduce(
                    wrd[:], t1p[:].rearrange("p (t q c) -> p t q c",
                                             q=WQ, c=NS),
                    mybir.AxisListType.X, Alu.max)
                nc.vector.tensor_scalar(wrd[:], wrd[:], 1.0, None,
                                        op0=Alu.min)
                # broadcast back over ch and transpose back on PE
                wexp = pools["wb"].tile([C, ln], f32, tag="wbx",
                                        padded_shape=[C, 1024])
                nc.scalar.activation(
                    wexp[:].rearrange("p (t q c) -> p t q c", q=WQ, c=NS),
                    wrd[:].rearrange("p (t q) -> p t q", q=WQ).unsqueeze(-1)
                    .broadcast_to((C, nt, WQ, NS)),
                    Act.Copy)
                wbp = pools["ps2"].tile([C, ln], f32, tag="ps2", space="PSUM",
                                        padded_shape=[C, 1024])
                for t in range(nt):
                    ts = slice(t * 128, (t + 1) * 128)
                    nc.tensor.transpose(wbp[:, ts], wexp[:, ts], ident[:])
                tt = pools["tb"].tile([C, ln], f32, tag="tb",
                                      padded_shape=[C, 1024])
                nc.vector.tensor_tensor(tt[:], wbp[:], dts[i][:, dsl],
                                        Alu.mult)
                rf = pools["rf"].tile([C, ln], f32, tag="rf",
                                      padded_shape=[C, 1024])
                nc.gpsimd.tensor_tensor(rf[:], tt[:], selts[i][:, dsl],
                                        Alu.add)
                nc.sync.dma_start(ref[:, sl], rf[:])
    nc.compile()
    return nc


# disparity ramp: jnp.linspace(0.1, 1.0, 96, dtype=float32) values
def _disp_f32():
    return np.linspace(0.1, 1.0, H).astype(np.float32)


def _select_channels(rows_sum_f64, ssq_f64):
    """Host-side ranking. rows_sum_f64: (C, H) summed over cores/batches,
    ssq_f64: (C,)."""
    disp = _disp_f32().astype(np.float64)
    depth = 1.0 - disp
    n_rep = B * W  # each h value appears B*W times in the full flattened vec
    dot_disp = rows_sum_f64 @ disp
    dot_depth = rows_sum_f64 @ depth
    vn_disp = np.sqrt(n_rep * (disp @ disp))
    vn_depth = np.sqrt(n_rep * (depth @ depth))
    sn = np.maximum(np.sqrt(ssq_f64), EPS)
    cos_disp = dot_disp / (sn * vn_disp)
    cos_depth = dot_depth / (sn * vn_depth)
    disp_idx = np.argsort(-cos_disp, kind="stable")[:NSEL]
    depth_idx = np.argsort(-cos_depth, kind="stable")[:NSEL]
    return np.concatenate([disp_idx, depth_idx])


def _pack_phase2_inputs(input_features, dynamic_masks, idx):
    """Pack selected channels and mask into the per-core (128, 7680) device
    layout: partition p = wq*32 + ch, free t = w'*96 + (95 - h)."""
    sel = input_features[:, idx]                       # (B, 32, H, W)
    sel_t = sel[:, :, ::-1, :].transpose(0, 1, 3, 2)   # (B, 32, W, Hrev)
    sel_p = np.ascontiguousarray(
        sel_t.reshape(B, NS, WQ, WPQ, H).transpose(0, 2, 1, 3, 4)
    ).reshape(B, C, S2)

    m_r = (dynamic_masks[:, ::-1, :] != 0).astype(np.uint8)  # (B, Hrev, W)
    m_r = m_r.copy()
    m_r[:, 0, :] = 0                # force reset at each column's bottom row
    m_t = m_r.transpose(0, 2, 1)    # (B, W, Hrev)
    m_q = np.ascontiguousarray(m_t).reshape(B, WQ, S2)
    m_big = np.broadcast_to(m_q[:, :, None, :], (B, WQ, NS, S2))
    m_big = np.ascontiguousarray(m_big).reshape(B, C, S2)
    return sel_p, m_big


def _unpack_refined(ref_stack):
    """(B, 128, 7680) device layout -> (B, 32, H, W)."""
    r = ref_stack.reshape(B, WQ, NS, WPQ, H).transpose(0, 2, 1, 3, 4)
    r = r.reshape(B, NS, W, H).transpose(0, 1, 3, 2)   # (B, 32, Hrev, W)
    return r[:, :, ::-1, :]


def _get_runners():
    if "run1" not in _cache:
        nc1 = build_phase1()
        _cache["run1"] = _runner(nc1, N_CORES)
        nc2 = build_phase2()
        _cache["run2"] = _runner(nc2, N_CORES)
    return _cache["run1"], _cache["run2"]


def _max_masked_run(dynamic_masks):
    """Longest run of consecutive masked rows in any column."""
    m = (dynamic_masks != 0)
    best = np.zeros((B, W), dtype=np.int32)
    cur = np.zeros((B, W), dtype=np.int32)
    for h in range(H - 1, -1, -1):
        cur = np.where(m[:, h, :], cur + 1, 0)
        best = np.maximum(best, cur)
    return int(best.max())


def kernel(input_features, dynamic_masks):
    input_features = np.asarray(input_features, dtype=np.float32)
    dynamic_masks = np.asarray(dynamic_masks)
    run1, run2 = _get_runners()

    # Phase 1: per-channel reductions on device
    in_maps1 = [
        {"x": input_features[b].reshape(C, HW)} for b in range(B)
    ]
    outs1 = run1(in_maps1)
    rows_sum = np.zeros((C, H), dtype=np.float64)
    ssq = np.zeros((C,), dtype=np.float64)
    for o in outs1:
        rows_sum += o["rows"].astype(np.float64)
        ssq += o["ssq"].astype(np.float64).sum(axis=1)
    idx = _select_channels(rows_sum, ssq)

    # the single-scan propagation is exact iff no masked run >= 33
    assert _max_masked_run(dynamic_masks) <= 32, (
        "masked run of >= 33 rows: single-scan shortcut invalid for this input"
    )

    # Phase 2: propagation + blend on device
    sel_p, m_big = _pack_phase2_inputs(input_features, dynamic_masks, idx)
    in_maps2 = [{"sel": sel_p[b], "msk": m_big[b]} for b in range(B)]
    outs2 = run2(in_maps2)
    ref_stack = np.stack([o["ref"] for o in outs2])
    refined = _unpack_refined(ref_stack)

    out = input_features.copy()
    out[:, idx] = refined
    return out



# revision 4
# speedup vs baseline: 1.5865x; 1.5865x over previous
"""Trainium2 Bass kernel for nn_GroundPropagation.

Structure (8 NeuronCores, batch-parallel, one batch element per core):

Phase 1 (device): channel-selection statistics over s = sigmoid(x).
  The host ships x in fp16, TRANSPOSED (positions on partitions, channels
  in the free dim, 240 blocks of 128 positions). Per block the device
  computes sigmoid on ACT, s^2 on DVE (fp16 2x mode), and one accumulating
  PE matmul with a tiny 2-column stationary [disp_pos, 1] against the
  moving [s | s^2] pair, yielding per-channel [sum(s*disp), sum(s)] and
  [_, sum(s^2)] in PSUM. Host combines the per-core f32 partials in f64
  and ranks channels by cosine similarity (top-16 disp + top-16 depth).

Phase 2 (device): for the 32 selected channels the 32-iteration masked
  propagation collapses to one bottom-up first-order recurrence per
  column. The host precomputes g = m * (sel_below - sel) so a single
  tensor_tensor_scan directly produces d = prop - sel:
      d[t] = m[t] * d[t-1] + g[t]
  (exact iff no column has >= 33 consecutive masked rows; checked on
  host). Then per (b,ch) m_clip = CLIP * max|d| (DVE reduce + tiny
  cross-partition max via PE transposes), w_px = |d| / m_clip on ACT,
  per-pixel max over the 32 channels via fp16 PE transposes + DVE
  reduce, and refined = sel + w * d, written back as fp16. Host scatters
  the 32 channels into the full f32 tensor.
"""

import sys

sys.path.insert(0, "/opt/trn_rl_repo")

import numpy as np

B, C, H, W = 8, 128, 96, 320
HW = H * W                  # 30720
NSEL = 16
NS = 2 * NSEL               # 32 selected channels
CLIP = 0.3
EPS = 1e-6
N_CORES = 8

# phase 1: transposed layout, 240 blocks of 128 positions
NBLK = HW // 128            # 240
P1C = 8                     # dma/compute chunks
BPC = NBLK // P1C           # 30 blocks per chunk

# phase 2: partition p = wq*32 + ch, free = (col, h_rev)
WQ = 4
WPQ = W // WQ               # 80 columns per quarter
S2 = WPQ * H                # 7680 free elems per partition
P2C = 4                     # stage chunks (20 columns each)
CH2 = S2 // P2C             # 1920
NB2 = CH2 // 128            # 15 transpose blocks per chunk

_cache = {}


def _runner(nc, n_cores):
    """Build a cached jitted callable for this Bass program via PJRT
    (mirrors concourse.bass2jax.run_bass_via_pjrt, but reusable)."""
    import jax
    from concourse import mybir
    from concourse.bass2jax import (
        _bass_exec_p,
        install_neuronx_cc_hook,
        partition_id_tensor,
    )
    from jax.sharding import Mesh, PartitionSpec
    from jax.experimental.shard_map import shard_map

    install_neuronx_cc_hook()
    partition_name = nc.partition_id_tensor.name if nc.partition_id_tensor else None

    in_names, out_names, out_avals = [], [], []
    for alloc in nc.m.functions[0].allocations:
        if not isinstance(alloc, mybir.MemoryLocationSet):
            continue
        name = alloc.memorylocations[0].name
        if alloc.kind == "ExternalInput":
            if name != partition_name:
                in_names.append(name)
        elif alloc.kind == "ExternalOutput":
            out_names.append(name)
            out_avals.append(
                jax.core.ShapedArray(
                    tuple(alloc.tensor_shape), mybir.dt.np(alloc.dtype)
                )
            )
    n_params = len(in_names)
    n_outs = len(out_avals)
    all_names = in_names + out_names + ([partition_name] if partition_name else [])
    donate = tuple(range(n_params, n_params + n_outs))

    def _body(*args):
        operands = list(args)
        if partition_name is not None:
            operands.append(partition_id_tensor())
        outs = _bass_exec_p.bind(
            *operands,
            out_avals=tuple(out_avals),
            in_names=tuple(all_names),
            out_names=tuple(out_names),
            lowering_input_output_aliases=(),
            sim_require_finite=True,
            sim_require_nnan=True,
            nc=nc,
        )
        return tuple(outs)

    devices = jax.devices()[:n_cores]
    mesh = Mesh(np.asarray(devices), ("core",))
    in_specs = (PartitionSpec("core"),) * (n_params + n_outs)
    out_specs = (PartitionSpec("core"),) * n_outs
    sharded = jax.jit(
        shard_map(
            _body, mesh=mesh, in_specs=in_specs, out_specs=out_specs, check_rep=False
        ),
        donate_argnums=donate,
        keep_unused=True,
    )

    def run(in_maps):
        concat_in = [
            np.concatenate([np.asarray(m[name]) for m in in_maps], axis=0)
            for name in in_names
        ]
        zeros = [
            np.zeros((n_cores * a.shape[0], *a.shape[1:]), a.dtype) for a in out_avals
        ]
        out_arrs = sharded(*concat_in, *zeros)
        return [
            {
                name: np.asarray(out_arrs[i]).reshape(
                    n_cores, *out_avals[i].shape
                )[c]
                for i, name in enumerate(out_names)
            }
            for c in range(n_cores)
        ]

    return run


def build_phase1():
    from contextlib import ExitStack

    import concourse.tile as tile
    from concourse import bacc, mybir

    f32 = mybir.dt.float32
    f16 = mybir.dt.float16
    Act = mybir.ActivationFunctionType
    Alu = mybir.AluOpType
    nc = bacc.Bacc("TRN2", target_bir_lowering=False, debug=False,
                   num_devices=N_CORES)
    xt = nc.dram_tensor("xt", (128, NBLK * 128), f16, kind="ExternalInput").ap()
    vb = nc.dram_tensor("vb", (128, NBLK * 2), f16, kind="ExternalInput").ap()
    acc = nc.dram_tensor("acc", (2, 256), f32, kind="ExternalOutput").ap()

    with tile.TileContext(nc) as tc, ExitStack() as ctx:
        px = ctx.enter_context(tc.tile_pool(name="px", bufs=3))
        ps = ctx.enter_context(tc.tile_pool(name="ps", bufs=3))
        psm = ctx.enter_context(tc.tile_pool(name="psm", bufs=1))
        pps = ctx.enter_context(tc.tile_pool(name="pps", bufs=1, space="PSUM"))

        vbt = psm.tile([128, NBLK * 2], f16)
        nc.sync.dma_start(vbt[:], vb[:])
        accp = pps.tile([2, 256], f32, space="PSUM")

        for i in range(P1C):
            fs = slice(i * BPC * 128, (i + 1) * BPC * 128)
            xtt = px.tile([128, BPC * 128], f16, tag="x")
            nc.sync.dma_start(xtt[:], xt[:, fs])
            # s and s^2 interleaved per block: (p, blk, {s,s2}, 128)
            st2 = ps.tile([128, BPC * 256], f16, tag="s")
            sview = st2[:].rearrange("p (b two c) -> p b two c", two=2, c=128)
            nc.scalar.activation(
                sview[:, :, 0, :],
                xtt[:].rearrange("p (b c) -> p b c", c=128),
                Act.Sigmoid,
            )
            nc.vector.tensor_tensor(
                sview[:, :, 1, :], sview[:, :, 0, :], sview[:, :, 0, :],
                Alu.mult,
            )
            for b in range(BPC):
                blk = i * BPC + b
                nc.tensor.matmul(
                    accp[:],
                    vbt[:, 2 * blk:2 * blk + 2],
                    st2[:, b * 256:(b + 1) * 256],
                    start=(blk == 0),
                    stop=(blk == NBLK - 1),
                )
        out_sb = psm.tile([2, 256], f32)
        nc.vector.tensor_copy(out_sb[:], accp[:])
        nc.sync.dma_start(acc[:], out_sb[:])
    nc.compile()
    return nc


def build_phase2():
    from contextlib import ExitStack

    import concourse.tile as tile
    from concourse import bacc, mybir
    from concourse.masks import make_identity

    f32 = mybir.dt.float32
    f16 = mybir.dt.float16
    u8 = mybir.dt.uint8
    Alu = mybir.AluOpType
    Act = mybir.ActivationFunctionType
    nc = bacc.Bacc("TRN2", target_bir_lowering=False, debug=False,
                   num_devices=N_CORES)
    sel = nc.dram_tensor("sel", (C, S2), f16, kind="ExternalInput").ap()
    gg = nc.dram_tensor("gg", (C, S2), f16, kind="ExternalInput").ap()
    msk = nc.dram_tensor("msk", (C, S2), u8, kind="ExternalInput").ap()
    ref = nc.dram_tensor("ref", (C, S2), f16, kind="ExternalOutput").ap()

    with tile.TileContext(nc) as tc, ExitStack() as ctx:
        pools = {}
        for name, bufs in [("sel", P2C + 1), ("g", 3), ("m", 3),
                           ("u", P2C + 1), ("wpx", 3), ("wexp", 3),
                           ("t", 3), ("rf", 3), ("sm", 1)]:
            pools[name] = ctx.enter_context(tc.tile_pool(name=name, bufs=bufs))
        pT = ctx.enter_context(tc.tile_pool(name="pT", bufs=2, space="PSUM"))
        pW = ctx.enter_context(tc.tile_pool(name="pW", bufs=1, space="PSUM"))
        pS = ctx.enter_context(tc.tile_pool(name="pS", bufs=1, space="PSUM"))
        psm = pools["sm"]

        ident = psm.tile([C, C], f16)
        make_identity(nc, ident[:])
        ident1 = psm.tile([1, 1], f32)
        nc.vector.memset(ident1[:], 1.0)

        mxp = psm.tile([C, P2C], f32)
        mcq = psm.tile([1, NS], f32)
        den = psm.tile([1, NS], f32)
        zc = psm.tile([1, NS], f32)
        rc1 = psm.tile([1, NS], f32)
        rcr = psm.tile([1, C], f32)
        rcp = psm.tile([C, 1], f32)

        selts, uts = [], []
        # ---- stage I: scan -> d, per-chunk abs-max ----
        for i in range(P2C):
            sl = slice(i * CH2, (i + 1) * CH2)
            mt = pools["m"].tile([C, CH2], u8, tag="m")
            nc.sync.dma_start(mt[:], msk[:, sl])
            gt = pools["g"].tile([C, CH2], f16, tag="g")
            nc.sync.dma_start(gt[:], gg[:, sl])
            selt = pools["sel"].tile([C, CH2], f16, tag="sel")
            nc.sync.dma_start(selt[:], sel[:, sl])
            ut = pools["u"].tile([C, CH2], f16, tag="u")
            nc.vector.tensor_tensor_scan(ut[:], mt[:], gt[:], 0.0,
                                         op0=Alu.mult, op1=Alu.add)
            nc.vector.tensor_reduce(
                mxp[:, i:i + 1], ut[:], mybir.AxisListType.X, Alu.max,
                apply_absolute_value=True)
            selts.append(selt)
            uts.append(ut)

        # ---- barrier: m_clip per channel -> 1/m_clip as (128,1) column ----
        mxr = psm.tile([C, 1], f16)
        nc.vector.tensor_reduce(mxr[:], mxp[:], mybir.AxisListType.X, Alu.max)
        trow = pS.tile([1, C], f16, space="PSUM")
        nc.tensor.transpose(trow[:], mxr[:], ident[:])
        # max over the 4 w-quarters for each channel
        nc.vector.tensor_reduce(
            mcq[:], trow[:].rearrange("o (q c) -> o c q", q=WQ),
            mybir.AxisListType.X, Alu.max)
        nc.vector.tensor_scalar(zc[:], mcq[:], 0.0, None, op0=Alu.is_equal)
        nc.vector.scalar_tensor_tensor(
            den[:], mcq[:], CLIP, zc[:], op0=Alu.mult, op1=Alu.add)
        nc.vector.reciprocal(rc1[:], den[:])
        nc.vector.tensor_copy(
            rcr[:].rearrange("o (q c) -> o q c", q=WQ),
            rc1[:].unsqueeze(1).broadcast_to((1, WQ, NS)))
        rcpp = pS.tile([C, 1], f32, space="PSUM")
        nc.tensor.matmul(rcpp[:], rcr[:], ident1[:], is_transpose=True)
        nc.scalar.copy(rcp[:], rcpp[:])

        # ---- stage II: w_px, channel max via PE transposes, blend ----
        for i in range(P2C):
            sl = slice(i * CH2, (i + 1) * CH2)
            wpx = pools["wpx"].tile([C, CH2], f16, tag="wpx")
            nc.scalar.activation(wpx[:], uts[i][:], Act.Abs, scale=rcp[:])
            wT = pT.tile([C, CH2], f16, tag="wT", space="PSUM")
            for t in range(NB2):
                ts = slice(t * 128, (t + 1) * 128)
                nc.tensor.transpose(wT[:, ts], wpx[:, ts], ident[:])
            wrd = psm.tile([C, NB2 * WQ], f16, tag=f"wrd{i}")
            nc.vector.tensor_reduce(
                wrd[:], wT[:].rearrange("p (t q c) -> p t q c", q=WQ, c=NS),
                mybir.AxisListType.X, Alu.max)
            nc.vector.tensor_scalar(wrd[:], wrd[:], 1.0, None, op0=Alu.min)
            wexp = pools["wexp"].tile([C, CH2], f16, tag="wexp")
            nc.scalar.activation(
                wexp[:].rearrange("p (t q c) -> p t q c", q=WQ, c=NS),
                wrd[:].rearrange("p (t q) -> p t q", q=WQ).unsqueeze(-1)
                .broadcast_to((C, NB2, WQ, NS)),
                Act.Copy)
            wB = pW.tile([C, CH2], f16, tag="wB", space="PSUM")
            for t in range(NB2):
                ts = slice(t * 128, (t + 1) * 128)
                nc.tensor.transpose(wB[:, ts], wexp[:, ts], ident[:])
            tt = pools["t"].tile([C, CH2], f16, tag="t")
            nc.vector.tensor_tensor(tt[:], wB[:], uts[i][:], Alu.mult)
            rf = pools["rf"].tile([C, CH2], f16, tag="rf")
            nc.gpsimd.tensor_tensor(rf[:], tt[:], selts[i][:], Alu.add)
            nc.sync.dma_start(ref[:, sl], rf[:])
    nc.compile()
    return nc


def _disp16():
    return np.linspace(0.1, 1.0, H).astype(np.float32).astype(np.float16)


def _pack_phase1(x):
    """x (B,C,H,W) f32 -> xt (B,128,NBLK*128) f16 transposed-block layout,
    vb (128, NBLK*2) f16 stationary [disp, 1] per (pos_in, blk)."""
    xt = x.transpose(0, 2, 3, 1).reshape(B, NBLK, 128, C)
    xt = np.ascontiguousarray(xt.transpose(0, 2, 1, 3)).reshape(B, 128, NBLK * C)
    xt = xt.astype(np.float16)

    disp_pos = np.repeat(_disp16(), W)                      # (HW,) f16
    vb = np.empty((NBLK, 128, 2), np.float16)
    vb[:, :, 0] = disp_pos.reshape(NBLK, 128)
    vb[:, :, 1] = 1.0
    vb = np.ascontiguousarray(vb.transpose(1, 0, 2)).reshape(128, NBLK * 2)
    return xt, vb


def _select_channels(acc_sum):
    """acc_sum: f64 (2,256) summed over cores. Rank channels by cosine
    similarity against the disp/depth ramps."""
    dot_disp = acc_sum[0, :C]
    tot = acc_sum[1, :C]
    ssq = acc_sum[1, C:]
    dot_depth = tot - dot_disp

    d16 = _disp16().astype(np.float64)
    n_rep = B * W
    vn_disp = np.sqrt(n_rep * (d16 @ d16))
    vn_depth = np.sqrt(n_rep * ((1.0 - d16) @ (1.0 - d16)))
    sn = np.maximum(np.sqrt(ssq), EPS)
    cos_disp = dot_disp / (sn * vn_disp)
    cos_depth = dot_depth / (sn * vn_depth)
    disp_idx = np.argsort(-cos_disp, kind="stable")[:NSEL]
    depth_idx = np.argsort(-cos_depth, kind="stable")[:NSEL]
    return np.concatenate([disp_idx, depth_idx])


def _pack_p2_layout(a):
    """(B, NS, Hrev, W) -> (B, 128, S2) with p = wq*32+ch, free = (col, h)."""
    at = a.transpose(0, 1, 3, 2)                            # (B, NS, W, Hrev)
    ap = np.ascontiguousarray(
        at.reshape(B, NS, WQ, WPQ, H).transpose(0, 2, 1, 3, 4)
    ).reshape(B, C, S2)
    return ap


def _pack_phase2(x, dynamic_masks, idx):
    """Pack sel, g = m*(sel_below - sel), and mask into the per-core
    (128, S2) device layout (fp16 / u8)."""
    sel = x[:, idx]                                         # (B, NS, H, W) f32
    sel_r = sel[:, :, ::-1, :]                              # bottom-up
    m_r = (dynamic_masks[:, ::-1, :] != 0)
    m_r = m_r.copy()
    m_r[:, 0, :] = False               # bottom row never pulls
    g_r = np.zeros_like(sel_r)
    g_r[:, :, 1:] = sel_r[:, :, :-1] - sel_r[:, :, 1:]
    g_r *= m_r[:, None].astype(np.float32)

    sel_p = _pack_p2_layout(sel_r).astype(np.float16)
    g_p = _pack_p2_layout(g_r).astype(np.float16)

    m_t = m_r.astype(np.uint8).transpose(0, 2, 1)           # (B, W, Hrev)
    m_q = np.ascontiguousarray(m_t).reshape(B, WQ, S2)
    m_big = np.broadcast_to(m_q[:, :, None, :], (B, WQ, NS, S2))
    m_p = np.ascontiguousarray(m_big).reshape(B, C, S2)
    return sel_p, g_p, m_p


def _unpack_refined(ref_stack):
    """(B, 128, S2) f16 device layout -> (B, NS, H, W) f32."""
    r = ref_stack.astype(np.float32)
    r = r.reshape(B, WQ, NS, WPQ, H).transpose(0, 2, 1, 3, 4)
    r = r.reshape(B, NS, W, H).transpose(0, 1, 3, 2)        # (B, NS, Hrev, W)
    return r[:, :, ::-1, :]


def _get_runners():
    if "run1" not in _cache:
        nc1 = build_phase1()
        _cache["run1"] = _runner(nc1, N_CORES)
        nc2 = build_phase2()
        _cache["run2"] = _runner(nc2, N_CORES)
    return _cache["run1"], _cache["run2"]


def _max_masked_run(dynamic_masks):
    m = (dynamic_masks != 0)
    best = np.zeros((B, W), dtype=np.int32)
    cur = np.zeros((B, W), dtype=np.int32)
    for h in range(H - 1, -1, -1):
        cur = np.where(m[:, h, :], cur + 1, 0)
        best = np.maximum(best, cur)
    return int(best.max())


def kernel(input_features, dynamic_masks):
    input_features = np.asarray(input_features, dtype=np.float32)
    dynamic_masks = np.asarray(dynamic_masks)
    run1, run2 = _get_runners()

    # Phase 1: per-channel similarity statistics on device
    xt, vb = _pack_phase1(input_features)
    in1 = [{"xt": xt[b], "vb": vb} for b in range(B)]
    outs1 = run1(in1)
    acc_sum = np.zeros((2, 256), np.float64)
    for o in outs1:
        acc_sum += o["acc"].astype(np.float64)
    idx = _select_channels(acc_sum)

    # single-scan propagation is exact iff no masked run >= 33
    assert _max_masked_run(dynamic_masks) <= 32, (
        "masked run of >= 33 rows: single-scan shortcut invalid for this input"
    )

    # Phase 2: propagation + blend on device
    sel_p, g_p, m_p = _pack_phase2(input_features, dynamic_masks, idx)
    in2 = [{"sel": sel_p[b], "gg": g_p[b], "msk": m_p[b]} for b in range(B)]
    outs2 = run2(in2)
    ref_stack = np.stack([o["ref"] for o in outs2])
    refined = _unpack_refined(ref_stack)

    out = input_features.copy()
    out[:, idx] = refined
    return out


# revision 13
# speedup vs baseline: 1.8304x; 1.1538x over previous
"""Trainium2 Bass kernel for nn_GroundPropagation.

Structure (8 NeuronCores, batch-parallel, one batch element per core):

Phase 1 (device): channel-selection statistics over s = sigmoid(x).
  The host ships x in fp16, TRANSPOSED (positions on partitions, channels
  in the free dim, 240 blocks of 128 positions). Per block the device
  computes sigmoid on ACT, s^2 on DVE (fp16 2x mode), and one accumulating
  PE matmul with a tiny 2-column stationary [disp_pos, 1] against the
  moving [s | s^2] pair, yielding per-channel [sum(s*disp), sum(s)] and
  [_, sum(s^2)] in PSUM. Host combines the per-core f32 partials in f64
  and ranks channels by cosine similarity (top-16 disp + top-16 depth).

Phase 2 (device): for the 32 selected channels the 32-iteration masked
  propagation collapses to one bottom-up first-order recurrence per
  column. The host precomputes g = m * (sel_below - sel) so a single
  tensor_tensor_scan directly produces d = prop - sel:
      d[t] = m[t] * d[t-1] + g[t]
  (exact iff no column has >= 33 consecutive masked rows; checked on
  host). Then per (b,ch) m_clip = CLIP * max|d| (DVE reduce + tiny
  cross-partition max via PE transposes), w_px = |d| / m_clip on ACT,
  per-pixel max over the 32 channels via fp16 PE transposes + DVE
  reduce, and refined = sel + w * d, written back as fp16. Host scatters
  the 32 channels into the full f32 tensor.
"""

import sys

sys.path.insert(0, "/opt/trn_rl_repo")

import numpy as np

B, C, H, W = 8, 128, 96, 320
HW = H * W                  # 30720
NSEL = 16
NS = 2 * NSEL               # 32 selected channels
CLIP = 0.3
EPS = 1e-6
N_CORES = 8

# phase 1: transposed layout, 240 blocks of 128 positions
NBLK = HW // 128            # 240
P1C = 8                     # dma/compute chunks
BPC = NBLK // P1C           # 30 blocks per chunk

# phase 2: partition p = wq*32 + ch, free = (col, h_rev)
WQ = 4
WPQ = W // WQ               # 80 columns per quarter
S2 = WPQ * H                # 7680 free elems per partition
P2C = 4                     # stage chunks (20 columns each)
CH2 = S2 // P2C             # 1920
NB2 = CH2 // 128            # 15 transpose blocks per chunk

_cache = {}


def _runner(nc, n_cores):
    """Build a cached jitted callable for this Bass program via PJRT
    (mirrors concourse.bass2jax.run_bass_via_pjrt, but reusable)."""
    import jax
    from concourse import mybir
    from concourse.bass2jax import (
        _bass_exec_p,
        install_neuronx_cc_hook,
        partition_id_tensor,
    )
    from jax.sharding import Mesh, PartitionSpec
    from jax.experimental.shard_map import shard_map

    install_neuronx_cc_hook()
    partition_name = nc.partition_id_tensor.name if nc.partition_id_tensor else None

    in_names, out_names, out_avals = [], [], []
    for alloc in nc.m.functions[0].allocations:
        if not isinstance(alloc, mybir.MemoryLocationSet):
            continue
        name = alloc.memorylocations[0].name
        if alloc.kind == "ExternalInput":
            if name != partition_name:
                in_names.append(name)
        elif alloc.kind == "ExternalOutput":
            out_names.append(name)
            out_avals.append(
                jax.core.ShapedArray(
                    tuple(alloc.tensor_shape), mybir.dt.np(alloc.dtype)
                )
            )
    n_params = len(in_names)
    n_outs = len(out_avals)
    all_names = in_names + out_names + ([partition_name] if partition_name else [])
    donate = tuple(range(n_params, n_params + n_outs))

    def _body(*args):
        operands = list(args)
        if partition_name is not None:
            operands.append(partition_id_tensor())
        outs = _bass_exec_p.bind(
            *operands,
            out_avals=tuple(out_avals),
            in_names=tuple(all_names),
            out_names=tuple(out_names),
            lowering_input_output_aliases=(),
            sim_require_finite=True,
            sim_require_nnan=True,
            nc=nc,
        )
        return tuple(outs)

    devices = jax.devices()[:n_cores]
    mesh = Mesh(np.asarray(devices), ("core",))
    in_specs = (PartitionSpec("core"),) * (n_params + n_outs)
    out_specs = (PartitionSpec("core"),) * n_outs
    sharded = jax.jit(
        shard_map(
            _body, mesh=mesh, in_specs=in_specs, out_specs=out_specs, check_rep=False
        ),
        donate_argnums=donate,
        keep_unused=True,
    )

    def run(in_maps):
        concat_in = [
            np.concatenate([np.asarray(m[name]) for m in in_maps], axis=0)
            for name in in_names
        ]
        zeros = [
            np.zeros((n_cores * a.shape[0], *a.shape[1:]), a.dtype) for a in out_avals
        ]
        out_arrs = sharded(*concat_in, *zeros)
        return [
            {
                name: np.asarray(out_arrs[i]).reshape(
                    n_cores, *out_avals[i].shape
                )[c]
                for i, name in enumerate(out_names)
            }
            for c in range(n_cores)
        ]

    return run


def build_phase1():
    from contextlib import ExitStack

    import concourse.tile as tile
    from concourse import bacc, mybir

    f32 = mybir.dt.float32
    f16 = mybir.dt.float16
    Act = mybir.ActivationFunctionType
    Alu = mybir.AluOpType
    nc = bacc.Bacc("TRN2", target_bir_lowering=False, debug=False,
                   num_devices=N_CORES)
    xt = nc.dram_tensor("xt", (128, NBLK * 128), f16, kind="ExternalInput").ap()
    vb = nc.dram_tensor("vb", (128, NBLK * 2), f16, kind="ExternalInput").ap()
    acc = nc.dram_tensor("acc", (2, 256), f32, kind="ExternalOutput").ap()

    with tile.TileContext(nc) as tc, ExitStack() as ctx:
        px = ctx.enter_context(tc.tile_pool(name="px", bufs=4))
        ps = ctx.enter_context(tc.tile_pool(name="ps", bufs=4))
        psm = ctx.enter_context(tc.tile_pool(name="psm", bufs=1))
        pps = ctx.enter_context(tc.tile_pool(name="pps", bufs=1, space="PSUM"))

        vbt = psm.tile([128, NBLK * 2], f16)
        nc.sync.dma_start(vbt[:], vb[:])
        accp = pps.tile([2, 256], f32, space="PSUM")

        # graded chunk sizes: small first so PE starts quickly
        sizes = [4, 8, 16, 24] + [31] * 6 + [2]
        assert sum(sizes) == NBLK
        blk0 = 0
        for i, nb in enumerate(sizes):
            fs = slice(blk0 * 128, (blk0 + nb) * 128)
            xtt = px.tile([128, nb * 128], f16, tag="x",
                          padded_shape=[128, 31 * 128])
            nc.sync.dma_start(xtt[:], xt[:, fs])
            # s and s^2 interleaved per block: (p, blk, {s,s2}, 128)
            st2 = ps.tile([128, nb * 256], f16, tag="s",
                          padded_shape=[128, 31 * 256])
            sview = st2[:].rearrange("p (b two c) -> p b two c", two=2, c=128)
            xview = xtt[:].rearrange("p (b c) -> p b c", c=128)
            halves = [(0, nb)] if nb <= 8 else [(0, nb // 2), (nb // 2, nb)]
            for h0, h1 in halves:
                nc.scalar.activation(
                    sview[:, h0:h1, 0, :], xview[:, h0:h1], Act.Sigmoid)
                nc.vector.tensor_tensor(
                    sview[:, h0:h1, 1, :], sview[:, h0:h1, 0, :],
                    sview[:, h0:h1, 0, :], Alu.mult,
                )
            for b in range(nb):
                blk = blk0 + b
                nc.tensor.matmul(
                    accp[:],
                    vbt[:, 2 * blk:2 * blk + 2],
                    st2[:, b * 256:(b + 1) * 256],
                    start=(blk == 0),
                    stop=(blk == NBLK - 1),
                )
            blk0 += nb
        out_sb = psm.tile([2, 256], f32)
        nc.vector.tensor_copy(out_sb[:], accp[:])
        nc.sync.dma_start(acc[:], out_sb[:])
    nc.compile()
    return nc


def build_phase2():
    from contextlib import ExitStack

    import concourse.tile as tile
    from concourse import bacc, mybir
    from concourse.masks import make_identity

    f32 = mybir.dt.float32
    f16 = mybir.dt.float16
    u8 = mybir.dt.uint8
    Alu = mybir.AluOpType
    Act = mybir.ActivationFunctionType
    nc = bacc.Bacc("TRN2", target_bir_lowering=False, debug=False,
                   num_devices=N_CORES)
    sel = nc.dram_tensor("sel", (C, S2), f16, kind="ExternalInput").ap()
    gg = nc.dram_tensor("gg", (C, S2), f16, kind="ExternalInput").ap()
    msk = nc.dram_tensor("msk", (C, S2), u8, kind="ExternalInput").ap()
    ref = nc.dram_tensor("ref", (C, S2), f16, kind="ExternalOutput").ap()

    # stage-I dma/scan chunks: graded so the first scan starts early
    SI = [480, 480, 960, 1920, 1920, 1920]
    assert sum(SI) == S2
    with tile.TileContext(nc) as tc, ExitStack() as ctx:
        pools = {}
        for name, bufs in [("sel", P2C + 1), ("g", 3), ("m", 3),
                           ("u", 1), ("wpx", 3), ("wexp", 3),
                           ("t", 3), ("rf", 3), ("sm", 1)]:
            pools[name] = ctx.enter_context(tc.tile_pool(name=name, bufs=bufs))
        pT = ctx.enter_context(tc.tile_pool(name="pT", bufs=2, space="PSUM"))
        pW = ctx.enter_context(tc.tile_pool(name="pW", bufs=2, space="PSUM"))
        psm = pools["sm"]

        ident = psm.tile([C, C], f16)
        make_identity(nc, ident[:])
        ident1 = psm.tile([1, 1], f32)
        nc.vector.memset(ident1[:], 1.0)

        mxp = psm.tile([C, len(SI)], f32)
        mcq = psm.tile([1, NS], f32)
        den = psm.tile([1, NS], f32)
        rc1 = psm.tile([1, NS], f32)
        rcr = psm.tile([1, C], f32)
        rcp = psm.tile([C, 1], f32)

        selts = []
        ubig = pools["u"].tile([C, S2], f16)
        # ---- stage I: scan -> d, per-chunk abs-max ----
        # masks+gaps first (they gate the scans); sel later (needed in II)
        off = 0
        for i, ln in enumerate(SI):
            sl = slice(off, off + ln)
            mt = pools["m"].tile([C, ln], u8, tag="m", bufs=3,
                                 padded_shape=[C, 1920])
            nc.sync.dma_start(mt[:], msk[:, sl])
            gt = pools["g"].tile([C, ln], f16, tag="g", bufs=3,
                                 padded_shape=[C, 1920])
            nc.sync.dma_start(gt[:], gg[:, sl])
            nc.vector.tensor_tensor_scan(ubig[:, sl], mt[:], gt[:], 0.0,
                                         op0=Alu.mult, op1=Alu.add)
            nc.vector.tensor_reduce(
                mxp[:, i:i + 1], ubig[:, sl], mybir.AxisListType.X, Alu.max,
                apply_absolute_value=True)
            off += ln
        for i in range(P2C):
            sl = slice(i * CH2, (i + 1) * CH2)
            selt = pools["sel"].tile([C, CH2], f16, tag="sel")
            nc.sync.dma_start(selt[:], sel[:, sl])
            selts.append(selt)

        # ---- barrier: m_clip per channel -> 1/m_clip as (128,1) column ----
        mxr = psm.tile([C, 1], f16)
        nc.vector.tensor_reduce(mxr[:], mxp[:], mybir.AxisListType.X, Alu.max)
        trow = pT.tile([1, C], f16, tag="wT", space="PSUM")
        nc.tensor.transpose(trow[:], mxr[:], ident[:])
        # max over the 4 w-quarters for each channel; clamp 0 -> tiny so the
        # reciprocal stays finite (a zero max means u==0 everywhere, so any
        # finite scale yields w_px = 0, matching the reference's m==0 -> 1)
        nc.vector.tensor_reduce(
            mcq[:], trow[:].rearrange("o (q c) -> o c q", q=WQ),
            mybir.AxisListType.X, Alu.max)
        nc.vector.tensor_scalar(den[:], mcq[:], CLIP, 1e-30,
                                op0=Alu.mult, op1=Alu.max)
        nc.vector.reciprocal(rc1[:], den[:])
        nc.vector.tensor_copy(
            rcr[:].rearrange("o (q c) -> o q c", q=WQ),
            rc1[:].unsqueeze(1).broadcast_to((1, WQ, NS)))
        rcpp = pW.tile([C, 1], f32, tag="wB", space="PSUM")
        nc.tensor.matmul(rcpp[:], rcr[:], ident1[:], is_transpose=True)
        nc.scalar.copy(rcp[:], rcpp[:])

        # ---- stage II: w_px, channel max via PE transposes, blend ----
        wpxs, wBs = [], []
        for i in range(P2C):
            sl = slice(i * CH2, (i + 1) * CH2)
            # w_px = d * (1/m_clip); sign dropped later by the |.| reduce
            wpx = pools["wpx"].tile([C, CH2], f16, tag="wpx", bufs=P2C)
            nc.vector.tensor_scalar(wpx[:], ubig[:, sl], rcp[:], None,
                                    op0=Alu.mult)
            wpxs.append(wpx)
        for i in range(P2C):
            sl = slice(i * CH2, (i + 1) * CH2)
            wpx = wpxs[i]
            wT = pT.tile([C, CH2], f16, tag="wT", space="PSUM")
            for t in range(NB2):
                ts = slice(t * 128, (t + 1) * 128)
                nc.tensor.transpose(wT[:, ts], wpx[:, ts], ident[:])
            wrd = psm.tile([C, NB2 * WQ], f16, tag=f"wrd{i}")
            nc.vector.tensor_reduce(
                wrd[:], wT[:].rearrange("p (t q c) -> p t q c", q=WQ, c=NS),
                mybir.AxisListType.X, Alu.max, apply_absolute_value=True)
            nc.vector.tensor_scalar(wrd[:], wrd[:], 1.0, None, op0=Alu.min)
            wexp = pools["wexp"].tile([C, CH2], f16, tag="wexp")
            nc.scalar.activation(
                wexp[:].rearrange("p (t q c) -> p t q c", q=WQ, c=NS),
                wrd[:].rearrange("p (t q) -> p t q", q=WQ).unsqueeze(-1)
                .broadcast_to((C, NB2, WQ, NS)),
                Act.Copy)
            wB = pW.tile([C, CH2], f16, tag="wB", space="PSUM")
            for t in range(NB2):
                ts = slice(t * 128, (t + 1) * 128)
                nc.tensor.transpose(wB[:, ts], wexp[:, ts], ident[:])
            wBs.append(wB)
        tts = []
        for i in range(P2C):
            tt = pools["t"].tile([C, CH2], f16, tag="t", bufs=P2C)
            nc.vector.tensor_tensor(
                tt[:], wBs[i][:], ubig[:, i * CH2:(i + 1) * CH2], Alu.mult)
            tts.append(tt)
        for i in range(P2C):
            sl = slice(i * CH2, (i + 1) * CH2)
            rf = pools["rf"].tile([C, CH2], f16, tag="rf", bufs=3)
            rf_eng = nc.gpsimd if i < 1 else nc.vector
            rf_eng.tensor_tensor(rf[:], tts[i][:], selts[i][:], Alu.add)
            nc.sync.dma_start(ref[:, sl], rf[:])
    nc.compile()
    return nc


def _disp16():
    return np.linspace(0.1, 1.0, H).astype(np.float32).astype(np.float16)


def _pack_phase1(x):
    """x (B,C,H,W) f32 -> xt (B,128,NBLK*128) f16 transposed-block layout,
    vb (128, NBLK*2) f16 stationary [disp, 1] per (pos_in, blk)."""
    xt = x.transpose(0, 2, 3, 1).reshape(B, NBLK, 128, C)
    xt = np.ascontiguousarray(xt.transpose(0, 2, 1, 3)).reshape(B, 128, NBLK * C)
    xt = xt.astype(np.float16)

    disp_pos = np.repeat(_disp16(), W)                      # (HW,) f16
    vb = np.empty((NBLK, 128, 2), np.float16)
    vb[:, :, 0] = disp_pos.reshape(NBLK, 128)
    vb[:, :, 1] = 1.0
    vb = np.ascontiguousarray(vb.transpose(1, 0, 2)).reshape(128, NBLK * 2)
    return xt, vb


def _select_channels(acc_sum):
    """acc_sum: f64 (2,256) summed over cores. Rank channels by cosine
    similarity against the disp/depth ramps."""
    dot_disp = acc_sum[0, :C]
    tot = acc_sum[1, :C]
    ssq = acc_sum[1, C:]
    dot_depth = tot - dot_disp

    d16 = _disp16().astype(np.float64)
    n_rep = B * W
    vn_disp = np.sqrt(n_rep * (d16 @ d16))
    vn_depth = np.sqrt(n_rep * ((1.0 - d16) @ (1.0 - d16)))
    sn = np.maximum(np.sqrt(ssq), EPS)
    cos_disp = dot_disp / (sn * vn_disp)
    cos_depth = dot_depth / (sn * vn_depth)
    disp_idx = np.argsort(-cos_disp, kind="stable")[:NSEL]
    depth_idx = np.argsort(-cos_depth, kind="stable")[:NSEL]
    return np.concatenate([disp_idx, depth_idx])


def _pack_p2_layout(a):
    """(B, NS, Hrev, W) -> (B, 128, S2) with p = wq*32+ch, free = (col, h)."""
    at = a.transpose(0, 1, 3, 2)                            # (B, NS, W, Hrev)
    ap = np.ascontiguousarray(
        at.reshape(B, NS, WQ, WPQ, H).transpose(0, 2, 1, 3, 4)
    ).reshape(B, C, S2)
    return ap


def _pack_phase2(x, dynamic_masks, idx):
    """Pack sel, g = m*(sel_below - sel), and mask into the per-core
    (128, S2) device layout (fp16 / u8)."""
    sel = x[:, idx]                                         # (B, NS, H, W) f32
    sel_r = sel[:, :, ::-1, :]                              # bottom-up
    m_r = (dynamic_masks[:, ::-1, :] != 0)
    m_r = m_r.copy()
    m_r[:, 0, :] = False               # bottom row never pulls
    g_r = np.zeros_like(sel_r)
    g_r[:, :, 1:] = sel_r[:, :, :-1] - sel_r[:, :, 1:]
    g_r *= m_r[:, None].astype(np.float32)

    sel_p = _pack_p2_layout(sel_r).astype(np.float16)
    g_p = _pack_p2_layout(g_r).astype(np.float16)

    m_t = m_r.astype(np.uint8).transpose(0, 2, 1)           # (B, W, Hrev)
    m_q = np.ascontiguousarray(m_t).reshape(B, WQ, S2)
    m_big = np.broadcast_to(m_q[:, :, None, :], (B, WQ, NS, S2))
    m_p = np.ascontiguousarray(m_big).reshape(B, C, S2)
    return sel_p, g_p, m_p


def _unpack_refined(ref_stack):
    """(B, 128, S2) f16 device layout -> (B, NS, H, W) f32."""
    r = ref_stack.astype(np.float32)
    r = r.reshape(B, WQ, NS, WPQ, H).transpose(0, 2, 1, 3, 4)
    r = r.reshape(B, NS, W, H).transpose(0, 1, 3, 2)        # (B, NS, Hrev, W)
    return r[:, :, ::-1, :]


def _get_runners():
    if "run1" not in _cache:
        nc1 = build_phase1()
        _cache["run1"] = _runner(nc1, N_CORES)
        nc2 = build_phase2()
        _cache["run2"] = _runner(nc2, N_CORES)
    return _cache["run1"], _cache["run2"]


def _max_masked_run(dynamic_masks):
    m = (dynamic_masks != 0)
    best = np.zeros((B, W), dtype=np.int32)
    cur = np.zeros((B, W), dtype=np.int32)
    for h in range(H - 1, -1, -1):
        cur = np.where(m[:, h, :], cur + 1, 0)
        best = np.maximum(best, cur)
    return int(best.max())


def kernel(input_features, dynamic_masks):
    input_features = np.asarray(input_features, dtype=np.float32)
    dynamic_masks = np.asarray(dynamic_masks)
    run1, run2 = _get_runners()

    # Phase 1: per-channel similarity statistics on device
    xt, vb = _pack_phase1(input_features)
    in1 = [{"xt": xt[b], "vb": vb} for b in range(B)]
    outs1 = run1(in1)
    acc_sum = np.zeros((2, 256), np.float64)
    for o in outs1:
        acc_sum += o["acc"].astype(np.float64)
    idx = _select_channels(acc_sum)

    # single-scan propagation is exact iff no masked run >= 33
    assert _max_masked_run(dynamic_masks) <= 32, (
        "masked run of >= 33 rows: single-scan shortcut invalid for this input"
    )

    # Phase 2: propagation + blend on device
    sel_p, g_p, m_p = _pack_phase2(input_features, dynamic_masks, idx)
    in2 = [{"sel": sel_p[b], "gg": g_p[b], "msk": m_p[b]} for b in range(B)]
    outs2 = run2(in2)
    ref_stack = np.stack([o["ref"] for o in outs2])
    refined = _unpack_refined(ref_stack)

    out = input_features.copy()
    out[:, idx] = refined
    return out


# revision 21
# speedup vs baseline: 1.8655x; 1.0191x over previous
"""Trainium2 Bass kernel for nn_GroundPropagation.

Structure (8 NeuronCores, batch-parallel, one batch element per core):

Phase 1 (device): channel-selection statistics over s = sigmoid(x).
  The host ships x in fp16, TRANSPOSED (positions on partitions, channels
  in the free dim, 240 blocks of 128 positions). Per block the device
  computes sigmoid on ACT, s^2 on DVE (fp16 2x mode), and one accumulating
  PE matmul with a tiny 2-column stationary [disp_pos, 1] against the
  moving [s | s^2] pair, yielding per-channel [sum(s*disp), sum(s)] and
  [_, sum(s^2)] in PSUM. Host combines the per-core f32 partials in f64
  and ranks channels by cosine similarity (top-16 disp + top-16 depth).

Phase 2 (device): for the 32 selected channels the 32-iteration masked
  propagation collapses to one bottom-up first-order recurrence per
  column. The host precomputes g = m * (sel_below - sel) so a single
  tensor_tensor_scan directly produces d = prop - sel:
      d[t] = m[t] * d[t-1] + g[t]
  (exact iff no column has >= 33 consecutive masked rows; checked on
  host). Then per (b,ch) m_clip = CLIP * max|d| (DVE reduce + tiny
  cross-partition max via PE transposes), w_px = |d| / m_clip on ACT,
  per-pixel max over the 32 channels via fp16 PE transposes + DVE
  reduce, and refined = sel + w * d, written back as fp16. Host scatters
  the 32 channels into the full f32 tensor.
"""

import sys

sys.path.insert(0, "/opt/trn_rl_repo")

import numpy as np

B, C, H, W = 8, 128, 96, 320
HW = H * W                  # 30720
NSEL = 16
NS = 2 * NSEL               # 32 selected channels
CLIP = 0.3
EPS = 1e-6
N_CORES = 8

# phase 1: transposed layout, 240 blocks of 128 positions
NBLK = HW // 128            # 240
P1C = 8                     # dma/compute chunks
BPC = NBLK // P1C           # 30 blocks per chunk

# phase 2: partition p = wq*32 + ch, free = (col, h_rev)
WQ = 4
WPQ = W // WQ               # 80 columns per quarter
S2 = WPQ * H                # 7680 free elems per partition
P2C = 4                     # stage chunks (20 columns each)
CH2 = S2 // P2C             # 1920
NB2 = CH2 // 128            # 15 transpose blocks per chunk

_cache = {}


def _runner(nc, n_cores):
    """Build a cached jitted callable for this Bass program via PJRT
    (mirrors concourse.bass2jax.run_bass_via_pjrt, but reusable)."""
    import jax
    from concourse import mybir
    from concourse.bass2jax import (
        _bass_exec_p,
        install_neuronx_cc_hook,
        partition_id_tensor,
    )
    from jax.sharding import Mesh, PartitionSpec
    from jax.experimental.shard_map import shard_map

    install_neuronx_cc_hook()
    partition_name = nc.partition_id_tensor.name if nc.partition_id_tensor else None

    in_names, out_names, out_avals = [], [], []
    for alloc in nc.m.functions[0].allocations:
        if not isinstance(alloc, mybir.MemoryLocationSet):
            continue
        name = alloc.memorylocations[0].name
        if alloc.kind == "ExternalInput":
            if name != partition_name:
                in_names.append(name)
        elif alloc.kind == "ExternalOutput":
            out_names.append(name)
            out_avals.append(
                jax.core.ShapedArray(
                    tuple(alloc.tensor_shape), mybir.dt.np(alloc.dtype)
                )
            )
    n_params = len(in_names)
    n_outs = len(out_avals)
    all_names = in_names + out_names + ([partition_name] if partition_name else [])
    donate = tuple(range(n_params, n_params + n_outs))

    def _body(*args):
        operands = list(args)
        if partition_name is not None:
            operands.append(partition_id_tensor())
        outs = _bass_exec_p.bind(
            *operands,
            out_avals=tuple(out_avals),
            in_names=tuple(all_names),
            out_names=tuple(out_names),
            lowering_input_output_aliases=(),
            sim_require_finite=True,
            sim_require_nnan=True,
            nc=nc,
        )
        return tuple(outs)

    devices = jax.devices()[:n_cores]
    mesh = Mesh(np.asarray(devices), ("core",))
    in_specs = (PartitionSpec("core"),) * (n_params + n_outs)
    out_specs = (PartitionSpec("core"),) * n_outs
    sharded = jax.jit(
        shard_map(
            _body, mesh=mesh, in_specs=in_specs, out_specs=out_specs, check_rep=False
        ),
        donate_argnums=donate,
        keep_unused=True,
    )

    def run(in_maps):
        concat_in = [
            np.concatenate([np.asarray(m[name]) for m in in_maps], axis=0)
            for name in in_names
        ]
        zeros = [
            np.zeros((n_cores * a.shape[0], *a.shape[1:]), a.dtype) for a in out_avals
        ]
        out_arrs = sharded(*concat_in, *zeros)
        return [
            {
                name: np.asarray(out_arrs[i]).reshape(
                    n_cores, *out_avals[i].shape
                )[c]
                for i, name in enumerate(out_names)
            }
            for c in range(n_cores)
        ]

    return run


def build_phase1():
    from contextlib import ExitStack

    import concourse.tile as tile
    from concourse import bacc, mybir

    f32 = mybir.dt.float32
    f16 = mybir.dt.float16
    Act = mybir.ActivationFunctionType
    Alu = mybir.AluOpType
    nc = bacc.Bacc("TRN2", target_bir_lowering=False, debug=False,
                   num_devices=N_CORES)
    xt = nc.dram_tensor("xt", (128, NBLK * 128), f16, kind="ExternalInput").ap()
    vb = nc.dram_tensor("vb", (128, NBLK * 2), f16, kind="ExternalInput").ap()
    acc = nc.dram_tensor("acc", (2, 512), f32, kind="ExternalOutput").ap()

    with tile.TileContext(nc) as tc, ExitStack() as ctx:
        px = ctx.enter_context(tc.tile_pool(name="px", bufs=4))
        ps = ctx.enter_context(tc.tile_pool(name="ps", bufs=4))
        psm = ctx.enter_context(tc.tile_pool(name="psm", bufs=1))
        pps = ctx.enter_context(tc.tile_pool(name="pps", bufs=1, space="PSUM"))

        vbt = psm.tile([128, NBLK * 2], f16)
        nc.sync.dma_start(vbt[:], vb[:])
        accp = pps.tile([2, 512], f32, space="PSUM")

        # graded chunk sizes: small first so PE starts quickly. The first
        # 192 blocks are "pure" (one image row per block, pair-aligned so
        # two blocks share one [disp_h, 1] stationary -> one 512-wide
        # matmul per pair); the last 48 blocks mix two rows each.
        sizes = [4, 8, 16, 24, 30, 30, 30, 30, 20] + [24, 16, 8]
        assert sum(sizes) == NBLK
        blk0 = 0
        for i, nb in enumerate(sizes):
            fs = slice(blk0 * 128, (blk0 + nb) * 128)
            xtt = px.tile([128, nb * 128], f16, tag="x",
                          padded_shape=[128, 30 * 128])
            nc.sync.dma_start(xtt[:], xt[:, fs])
            # s and s^2 interleaved per block: (p, blk, {s,s2}, 128)
            st2 = ps.tile([128, nb * 256], f16, tag="s",
                          padded_shape=[128, 30 * 256])
            sview = st2[:].rearrange("p (b two c) -> p b two c", two=2, c=128)
            xview = xtt[:].rearrange("p (b c) -> p b c", c=128)
            halves = [(0, nb)] if nb <= 8 else [(0, nb // 2), (nb // 2, nb)]
            for h0, h1 in halves:
                nc.scalar.activation(
                    sview[:, h0:h1, 0, :], xview[:, h0:h1], Act.Sigmoid)
                nc.vector.tensor_tensor(
                    sview[:, h0:h1, 1, :], sview[:, h0:h1, 0, :],
                    sview[:, h0:h1, 0, :], Alu.mult,
                )
            if blk0 + nb <= 192:
                for b in range(0, nb, 2):
                    blk = blk0 + b
                    nc.tensor.matmul(
                        accp[:],
                        vbt[:, 2 * blk:2 * blk + 2],
                        st2[:, b * 256:(b + 2) * 256],
                        start=(blk == 0),
                        stop=False,
                    )
            else:
                for b in range(nb):
                    blk = blk0 + b
                    nc.tensor.matmul(
                        accp[:, :256],
                        vbt[:, 2 * blk:2 * blk + 2],
                        st2[:, b * 256:(b + 1) * 256],
                        start=False,
                        stop=(blk == NBLK - 1),
                    )
            blk0 += nb
        out_sb = psm.tile([2, 512], f32)
        nc.vector.tensor_copy(out_sb[:], accp[:])
        nc.sync.dma_start(acc[:], out_sb[:])
    nc.compile()
    return nc


def build_phase2():
    from contextlib import ExitStack

    import concourse.tile as tile
    from concourse import bacc, mybir
    from concourse.masks import make_identity

    f32 = mybir.dt.float32
    f16 = mybir.dt.float16
    u8 = mybir.dt.uint8
    Alu = mybir.AluOpType
    Act = mybir.ActivationFunctionType
    nc = bacc.Bacc("TRN2", target_bir_lowering=False, debug=False,
                   num_devices=N_CORES)
    sel = nc.dram_tensor("sel", (C, S2), f16, kind="ExternalInput").ap()
    gg = nc.dram_tensor("gg", (C, S2), f16, kind="ExternalInput").ap()
    msk = nc.dram_tensor("msk", (C, S2), u8, kind="ExternalInput").ap()
    ref = nc.dram_tensor("ref", (C, S2), f16, kind="ExternalOutput").ap()

    # stage-I dma/scan chunks: graded so the first scan starts early
    SI = [480, 1152, 3072, 2976]  # multiples of 96 (column-aligned)
    assert sum(SI) == S2
    with tile.TileContext(nc) as tc, ExitStack() as ctx:
        pools = {}
        for name, bufs in [("sel", P2C + 1), ("g", 3), ("m", 3),
                           ("u", 1), ("wpx", 3), ("wexp", 3),
                           ("t", 3), ("rf", 3), ("sm", 1)]:
            pools[name] = ctx.enter_context(tc.tile_pool(name=name, bufs=bufs))
        pT = ctx.enter_context(tc.tile_pool(name="pT", bufs=2, space="PSUM"))
        pW = ctx.enter_context(tc.tile_pool(name="pW", bufs=2, space="PSUM"))
        psm = pools["sm"]

        ident = psm.tile([C, C], f16)
        make_identity(nc, ident[:])
        ident1 = psm.tile([1, 1], f32)
        nc.vector.memset(ident1[:], 1.0)

        mxp = psm.tile([C, len(SI)], f32)
        mcq = psm.tile([1, NS], f32)
        den = psm.tile([1, NS], f32)
        rc1 = psm.tile([1, NS], f32)
        rcr = psm.tile([1, C], f32)
        rcp = psm.tile([C, 1], f32)

        selts = []
        ubig = pools["u"].tile([C, S2], f16)
        # ---- stage I: scan -> d, per-chunk abs-max ----
        # masks+gaps first (they gate the scans); sel later (needed in II)
        off = 0
        for i, ln in enumerate(SI):
            sl = slice(off, off + ln)
            mt = pools["m"].tile([C, ln], u8, tag="m", bufs=3,
                                 padded_shape=[C, 3072])
            nc.sync.dma_start(mt[:], msk[:, sl])
            gt = pools["g"].tile([C, ln], f16, tag="g", bufs=3,
                                 padded_shape=[C, 3072])
            nc.sync.dma_start(gt[:], gg[:, sl])
            nc.vector.tensor_tensor_scan(ubig[:, sl], mt[:], gt[:], 0.0,
                                         op0=Alu.mult, op1=Alu.add)
            nc.vector.tensor_reduce(
                mxp[:, i:i + 1], ubig[:, sl], mybir.AxisListType.X, Alu.max,
                apply_absolute_value=True)
            off += ln
        for i in range(P2C):
            sl = slice(i * CH2, (i + 1) * CH2)
            selt = pools["sel"].tile([C, CH2], f16, tag="sel")
            nc.sync.dma_start(selt[:], sel[:, sl])
            selts.append(selt)

        # ---- barrier: m_clip per channel -> 1/m_clip as (128,1) column ----
        mxr = psm.tile([C, 1], f16)
        nc.vector.tensor_reduce(mxr[:], mxp[:], mybir.AxisListType.X, Alu.max)
        trow = pT.tile([1, C], f16, tag="wT", space="PSUM")
        nc.tensor.transpose(trow[:], mxr[:], ident[:])
        # max over the 4 w-quarters for each channel; clamp 0 -> tiny so the
        # reciprocal stays finite (a zero max means u==0 everywhere, so any
        # finite scale yields w_px = 0, matching the reference's m==0 -> 1)
        nc.vector.tensor_reduce(
            mcq[:], trow[:].rearrange("o (q c) -> o c q", q=WQ),
            mybir.AxisListType.X, Alu.max)
        nc.vector.tensor_scalar(den[:], mcq[:], CLIP, 1e-30,
                                op0=Alu.mult, op1=Alu.max)
        nc.vector.reciprocal(rc1[:], den[:])
        nc.vector.tensor_copy(
            rcr[:].rearrange("o (q c) -> o q c", q=WQ),
            rc1[:].unsqueeze(1).broadcast_to((1, WQ, NS)))
        rcpp = pW.tile([C, 1], f32, tag="wB", space="PSUM")
        nc.tensor.matmul(rcpp[:], rcr[:], ident1[:], is_transpose=True)
        nc.scalar.copy(rcp[:], rcpp[:])

        # ---- stage II: w_px, channel max via PE transposes, blend ----
        wpxs, wBs = [], []
        for i in range(P2C):
            sl = slice(i * CH2, (i + 1) * CH2)
            # w_px = d * (1/m_clip); sign dropped later by the |.| reduce
            wpx = pools["wpx"].tile([C, CH2], f16, tag="wpx", bufs=P2C)
            nc.vector.tensor_scalar(wpx[:], ubig[:, sl], rcp[:], None,
                                    op0=Alu.mult)
            wpxs.append(wpx)
        for i in range(P2C):
            sl = slice(i * CH2, (i + 1) * CH2)
            wpx = wpxs[i]
            wT = pT.tile([C, CH2], f16, tag="wT", space="PSUM")
            for t in range(NB2):
                ts = slice(t * 128, (t + 1) * 128)
                nc.tensor.transpose(wT[:, ts], wpx[:, ts], ident[:])
            wrd = psm.tile([C, NB2 * WQ], f16, tag=f"wrd{i}")
            nc.vector.tensor_reduce(
                wrd[:], wT[:].rearrange("p (t q c) -> p t q c", q=WQ, c=NS),
                mybir.AxisListType.X, Alu.max, apply_absolute_value=True)
            nc.vector.tensor_scalar(wrd[:], wrd[:], 1.0, None, op0=Alu.min)
            wexp = pools["wexp"].tile([C, CH2], f16, tag="wexp")
            nc.scalar.activation(
                wexp[:].rearrange("p (t q c) -> p t q c", q=WQ, c=NS),
                wrd[:].rearrange("p (t q) -> p t q", q=WQ).unsqueeze(-1)
                .broadcast_to((C, NB2, WQ, NS)),
                Act.Copy)
            wB = pW.tile([C, CH2], f16, tag="wB", space="PSUM")
            for t in range(NB2):
                ts = slice(t * 128, (t + 1) * 128)
                nc.tensor.transpose(wB[:, ts], wexp[:, ts], ident[:])
            wBs.append(wB)
        tts = []
        for i in range(P2C):
            tt = pools["t"].tile([C, CH2], f16, tag="t", bufs=P2C)
            nc.vector.tensor_tensor(
                tt[:], wBs[i][:], ubig[:, i * CH2:(i + 1) * CH2], Alu.mult)
            tts.append(tt)
        for i in range(P2C):
            sl = slice(i * CH2, (i + 1) * CH2)
            rf = pools["rf"].tile([C, CH2], f16, tag="rf", bufs=3)
            if i < 1:
                nc.gpsimd.tensor_tensor(rf[:], tts[i][:], selts[i][:], Alu.add)
                nc.sync.dma_start(ref[:, sl], rf[:])
            elif i < P2C - 1:
                nc.vector.tensor_tensor(rf[:], tts[i][:], selts[i][:], Alu.add)
                nc.sync.dma_start(ref[:, sl], rf[:])
            else:
                hh = CH2 // 2
                nc.vector.tensor_tensor(rf[:, :hh], tts[i][:, :hh],
                                        selts[i][:, :hh], Alu.add)
                nc.sync.dma_start(ref[:, i * CH2:i * CH2 + hh], rf[:, :hh])
                nc.vector.tensor_tensor(rf[:, hh:], tts[i][:, hh:],
                                        selts[i][:, hh:], Alu.add)
                nc.sync.dma_start(ref[:, i * CH2 + hh:(i + 1) * CH2],
                                  rf[:, hh:])
    nc.compile()
    return nc


def _disp16():
    return np.linspace(0.1, 1.0, H).astype(np.float32).astype(np.float16)


def _blk_positions():
    """(NBLK, 128) flattened positions: blocks 0..191 hold one image row's
    w in [0,256) (two blocks per row, so a pair shares a constant disp);
    blocks 192..239 hold the w in [256,320) leftovers of two rows each."""
    pos = np.empty((NBLK, 128), np.int64)
    hh = np.arange(H)
    pure = (hh[:, None, None] * W
            + np.arange(256).reshape(2, 128)[None]).reshape(H * 2, 128)
    pos[:192] = pure
    mixed = (np.arange(H).reshape(48, 2)[:, :, None] * W
             + np.arange(256, 320)[None, None, :]).reshape(48, 128)
    pos[192:] = mixed
    return pos


def _pack_phase1(x):
    """x (B,C,H,W) f32 -> xt (B,128,NBLK*128) f16 transposed-block layout,
    vb (128, NBLK*2) f16 stationary [disp, 1] per (pos_in, blk)."""
    pos = _blk_positions()
    xflat = x.transpose(0, 2, 3, 1).reshape(B, HW, C)
    xt = xflat[:, pos.reshape(-1), :].reshape(B, NBLK, 128, C)
    xt = np.ascontiguousarray(xt.transpose(0, 2, 1, 3)).reshape(B, 128, NBLK * C)
    xt = xt.astype(np.float16)

    disp_pos = np.repeat(_disp16(), W)                      # (HW,) f16
    vb = np.empty((NBLK, 128, 2), np.float16)
    vb[:, :, 0] = disp_pos[pos]
    vb[:, :, 1] = 1.0
    vb = np.ascontiguousarray(vb.transpose(1, 0, 2)).reshape(128, NBLK * 2)
    return xt, vb


def _select_channels(acc_sum):
    """acc_sum: f64 (2,512) summed over cores (paired-matmul halves).
    Rank channels by cosine similarity against the disp/depth ramps."""
    half = acc_sum[:, :256] + acc_sum[:, 256:]
    dot_disp = half[0, :C]
    tot = half[1, :C]
    ssq = half[1, C:]
    dot_depth = tot - dot_disp

    d16 = _disp16().astype(np.float64)
    n_rep = B * W
    vn_disp = np.sqrt(n_rep * (d16 @ d16))
    vn_depth = np.sqrt(n_rep * ((1.0 - d16) @ (1.0 - d16)))
    sn = np.maximum(np.sqrt(ssq), EPS)
    cos_disp = dot_disp / (sn * vn_disp)
    cos_depth = dot_depth / (sn * vn_depth)
    disp_idx = np.argsort(-cos_disp, kind="stable")[:NSEL]
    depth_idx = np.argsort(-cos_depth, kind="stable")[:NSEL]
    return np.concatenate([disp_idx, depth_idx])


def _pack_p2_layout(a):
    """(B, NS, Hrev, W) -> (B, 128, S2) with p = wq*32+ch, free = (col, h)."""
    at = a.transpose(0, 1, 3, 2)                            # (B, NS, W, Hrev)
    ap = np.ascontiguousarray(
        at.reshape(B, NS, WQ, WPQ, H).transpose(0, 2, 1, 3, 4)
    ).reshape(B, C, S2)
    return ap


def _pack_phase2(x, dynamic_masks, idx):
    """Pack sel, g = m*(sel_below - sel), and mask into the per-core
    (128, S2) device layout (fp16 / u8)."""
    sel = x[:, idx]                                         # (B, NS, H, W) f32
    sel_r = sel[:, :, ::-1, :]                              # bottom-up
    m_r = (dynamic_masks[:, ::-1, :] != 0)
    m_r = m_r.copy()
    m_r[:, 0, :] = False               # bottom row never pulls
    g_r = np.zeros_like(sel_r)
    g_r[:, :, 1:] = sel_r[:, :, :-1] - sel_r[:, :, 1:]
    g_r *= m_r[:, None].astype(np.float32)

    sel_p = _pack_p2_layout(sel_r).astype(np.float16)
    g_p = _pack_p2_layout(g_r).astype(np.float16)

    m_t = m_r.astype(np.uint8).transpose(0, 2, 1)           # (B, W, Hrev)
    m_q = np.ascontiguousarray(m_t).reshape(B, WQ, S2)
    m_big = np.broadcast_to(m_q[:, :, None, :], (B, WQ, NS, S2))
    m_p = np.ascontiguousarray(m_big).reshape(B, C, S2)
    return sel_p, g_p, m_p


def _unpack_refined(ref_stack):
    """(B, 128, S2) f16 device layout -> (B, NS, H, W) f32."""
    r = ref_stack.astype(np.float32)
    r = r.reshape(B, WQ, NS, WPQ, H).transpose(0, 2, 1, 3, 4)
    r = r.reshape(B, NS, W, H).transpose(0, 1, 3, 2)        # (B, NS, Hrev, W)
    return r[:, :, ::-1, :]


def _get_runners():
    if "run1" not in _cache:
        nc1 = build_phase1()
        _cache["run1"] = _runner(nc1, N_CORES)
        nc2 = build_phase2()
        _cache["run2"] = _runner(nc2, N_CORES)
    return _cache["run1"], _cache["run2"]


def _max_masked_run(dynamic_masks):
    m = (dynamic_masks != 0)
    best = np.zeros((B, W), dtype=np.int32)
    cur = np.zeros((B, W), dtype=np.int32)
    for h in range(H - 1, -1, -1):
        cur = np.where(m[:, h, :], cur + 1, 0)
        best = np.maximum(best, cur)
    return int(best.max())


def kernel(input_features, dynamic_masks):
    input_features = np.asarray(input_features, dtype=np.float32)
    dynamic_masks = np.asarray(dynamic_masks)
    run1, run2 = _get_runners()

    # Phase 1: per-channel similarity statistics on device
    xt, vb = _pack_phase1(input_features)
    in1 = [{"xt": xt[b], "vb": vb} for b in range(B)]
    outs1 = run1(in1)
    acc_sum = np.zeros((2, 512), np.float64)
    for o in outs1:
        acc_sum += o["acc"].astype(np.float64)
    idx = _select_channels(acc_sum)

    # single-scan propagation is exact iff no masked run >= 33
    assert _max_masked_run(dynamic_masks) <= 32, (
        "masked run of >= 33 rows: single-scan shortcut invalid for this input"
    )

    # Phase 2: propagation + blend on device
    sel_p, g_p, m_p = _pack_phase2(input_features, dynamic_masks, idx)
    in2 = [{"sel": sel_p[b], "gg": g_p[b], "msk": m_p[b]} for b in range(B)]
    outs2 = run2(in2)
    ref_stack = np.stack([o["ref"] for o in outs2])
    refined = _unpack_refined(ref_stack)

    out = input_features.copy()
    out[:, idx] = refined
    return out


# revision 28
# speedup vs baseline: 1.8969x; 1.0168x over previous
"""Trainium2 Bass kernel for nn_GroundPropagation.

Structure (8 NeuronCores, batch-parallel, one batch element per core):

Phase 1 (device): channel-selection statistics over s = sigmoid(x).
  The host ships x in fp16, TRANSPOSED (positions on partitions, channels
  in the free dim, 240 blocks of 128 positions). Per block the device
  computes sigmoid on ACT, s^2 on DVE (fp16 2x mode), and one accumulating
  PE matmul with a tiny 2-column stationary [disp_pos, 1] against the
  moving [s | s^2] pair, yielding per-channel [sum(s*disp), sum(s)] and
  [_, sum(s^2)] in PSUM. Host combines the per-core f32 partials in f64
  and ranks channels by cosine similarity (top-16 disp + top-16 depth).

Phase 2 (device): for the 32 selected channels the 32-iteration masked
  propagation collapses to one bottom-up first-order recurrence per
  column. The host precomputes g = m * (sel_below - sel) so a single
  tensor_tensor_scan directly produces d = prop - sel:
      d[t] = m[t] * d[t-1] + g[t]
  (exact iff no column has >= 33 consecutive masked rows; checked on
  host). Then per (b,ch) m_clip = CLIP * max|d| (DVE reduce + tiny
  cross-partition max via PE transposes), w_px = |d| / m_clip on ACT,
  per-pixel max over the 32 channels via fp16 PE transposes + DVE
  reduce, and refined = sel + w * d, written back as fp16. Host scatters
  the 32 channels into the full f32 tensor.
"""

import sys

sys.path.insert(0, "/opt/trn_rl_repo")

import numpy as np

B, C, H, W = 8, 128, 96, 320
HW = H * W                  # 30720
NSEL = 16
NS = 2 * NSEL               # 32 selected channels
CLIP = 0.3
EPS = 1e-6
N_CORES = 8

# phase 1: transposed layout, 240 blocks of 128 positions
NBLK = HW // 128            # 240
P1C = 8                     # dma/compute chunks
BPC = NBLK // P1C           # 30 blocks per chunk

# phase 2: partition p = wq*32 + ch, free = (col, h_rev)
WQ = 4
WPQ = W // WQ               # 80 columns per quarter
S2 = WPQ * H                # 7680 free elems per partition
P2C = 4                     # stage chunks (20 columns each)
CH2 = S2 // P2C             # 1920
NB2 = CH2 // 128            # 15 transpose blocks per chunk

_cache = {}


def _runner(nc, n_cores):
    """Build a cached jitted callable for this Bass program via PJRT
    (mirrors concourse.bass2jax.run_bass_via_pjrt, but reusable)."""
    import jax
    from concourse import mybir
    from concourse.bass2jax import (
        _bass_exec_p,
        install_neuronx_cc_hook,
        partition_id_tensor,
    )
    from jax.sharding import Mesh, PartitionSpec
    from jax.experimental.shard_map import shard_map

    install_neuronx_cc_hook()
    partition_name = nc.partition_id_tensor.name if nc.partition_id_tensor else None

    in_names, out_names, out_avals = [], [], []
    for alloc in nc.m.functions[0].allocations:
        if not isinstance(alloc, mybir.MemoryLocationSet):
            continue
        name = alloc.memorylocations[0].name
        if alloc.kind == "ExternalInput":
            if name != partition_name:
                in_names.append(name)
        elif alloc.kind == "ExternalOutput":
            out_names.append(name)
            out_avals.append(
                jax.core.ShapedArray(
                    tuple(alloc.tensor_shape), mybir.dt.np(alloc.dtype)
                )
            )
    n_params = len(in_names)
    n_outs = len(out_avals)
    all_names = in_names + out_names + ([partition_name] if partition_name else [])
    donate = tuple(range(n_params, n_params + n_outs))

    def _body(*args):
        operands = list(args)
        if partition_name is not None:
            operands.append(partition_id_tensor())
        outs = _bass_exec_p.bind(
            *operands,
            out_avals=tuple(out_avals),
            in_names=tuple(all_names),
            out_names=tuple(out_names),
            lowering_input_output_aliases=(),
            sim_require_finite=True,
            sim_require_nnan=True,
            nc=nc,
        )
        return tuple(outs)

    devices = jax.devices()[:n_cores]
    mesh = Mesh(np.asarray(devices), ("core",))
    in_specs = (PartitionSpec("core"),) * (n_params + n_outs)
    out_specs = (PartitionSpec("core"),) * n_outs
    sharded = jax.jit(
        shard_map(
            _body, mesh=mesh, in_specs=in_specs, out_specs=out_specs, check_rep=False
        ),
        donate_argnums=donate,
        keep_unused=True,
    )

    def run(in_maps):
        concat_in = [
            np.concatenate([np.asarray(m[name]) for m in in_maps], axis=0)
            for name in in_names
        ]
        zeros = [
            np.zeros((n_cores * a.shape[0], *a.shape[1:]), a.dtype) for a in out_avals
        ]
        out_arrs = sharded(*concat_in, *zeros)
        return [
            {
                name: np.asarray(out_arrs[i]).reshape(
                    n_cores, *out_avals[i].shape
                )[c]
                for i, name in enumerate(out_names)
            }
            for c in range(n_cores)
        ]

    return run


def build_phase1():
    from contextlib import ExitStack

    import concourse.tile as tile
    from concourse import bacc, mybir

    f32 = mybir.dt.float32
    f16 = mybir.dt.float16
    Act = mybir.ActivationFunctionType
    Alu = mybir.AluOpType
    nc = bacc.Bacc("TRN2", target_bir_lowering=False, debug=False,
                   num_devices=N_CORES)
    xt = nc.dram_tensor("xt", (128, NBLK * 128), f16, kind="ExternalInput").ap()
    vb = nc.dram_tensor("vb", (128, NBLK * 2), f16, kind="ExternalInput").ap()
    acc = nc.dram_tensor("acc", (2, 512), f32, kind="ExternalOutput").ap()

    with tile.TileContext(nc) as tc, ExitStack() as ctx:
        px = ctx.enter_context(tc.tile_pool(name="px", bufs=4))
        ps = ctx.enter_context(tc.tile_pool(name="ps", bufs=4))
        psm = ctx.enter_context(tc.tile_pool(name="psm", bufs=1))
        pps = ctx.enter_context(tc.tile_pool(name="pps", bufs=1, space="PSUM"))

        vbt = psm.tile([128, NBLK * 2], f16)
        nc.sync.dma_start(vbt[:], vb[:])
        accp = pps.tile([2, 512], f32, space="PSUM")

        # graded chunk sizes: small first so PE starts quickly. The first
        # 192 blocks are "pure" (one image row per block, pair-aligned so
        # two blocks share one [disp_h, 1] stationary -> one 512-wide
        # matmul per pair); the last 48 blocks mix two rows each.
        sizes = [4, 8, 16, 24, 30, 30, 30, 30, 20] + [24, 16, 4, 4]
        assert sum(sizes) == NBLK
        blk0 = 0
        for i, nb in enumerate(sizes):
            fs = slice(blk0 * 128, (blk0 + nb) * 128)
            xtt = px.tile([128, nb * 128], f16, tag="x",
                          padded_shape=[128, 30 * 128])
            nc.sync.dma_start(xtt[:], xt[:, fs])
            # s and s^2 interleaved per block: (p, blk, {s,s2}, 128)
            st2 = ps.tile([128, nb * 256], f16, tag="s",
                          padded_shape=[128, 30 * 256])
            sview = st2[:].rearrange("p (b two c) -> p b two c", two=2, c=128)
            xview = xtt[:].rearrange("p (b c) -> p b c", c=128)
            halves = [(0, nb)] if nb <= 8 else [(0, nb // 2), (nb // 2, nb)]
            for h0, h1 in halves:
                nc.scalar.activation(
                    sview[:, h0:h1, 0, :], xview[:, h0:h1], Act.Sigmoid)
                nc.vector.tensor_tensor(
                    sview[:, h0:h1, 1, :], sview[:, h0:h1, 0, :],
                    sview[:, h0:h1, 0, :], Alu.mult,
                )
            if blk0 + nb <= 192:
                for b in range(0, nb, 2):
                    blk = blk0 + b
                    nc.tensor.matmul(
                        accp[:],
                        vbt[:, 2 * blk:2 * blk + 2],
                        st2[:, b * 256:(b + 2) * 256],
                        start=(blk == 0),
                        stop=False,
                    )
            else:
                for b in range(nb):
                    blk = blk0 + b
                    nc.tensor.matmul(
                        accp[:, :256],
                        vbt[:, 2 * blk:2 * blk + 2],
                        st2[:, b * 256:(b + 1) * 256],
                        start=False,
                        stop=(blk == NBLK - 1),
                    )
            blk0 += nb
        out_sb = psm.tile([2, 512], f32)
        nc.vector.tensor_copy(out_sb[:], accp[:])
        nc.sync.dma_start(acc[:], out_sb[:])
    nc.compile()
    return nc


def build_phase2():
    from contextlib import ExitStack

    import concourse.tile as tile
    from concourse import bacc, mybir
    from concourse.masks import make_identity

    f32 = mybir.dt.float32
    f16 = mybir.dt.float16
    u8 = mybir.dt.uint8
    Alu = mybir.AluOpType
    Act = mybir.ActivationFunctionType
    nc = bacc.Bacc("TRN2", target_bir_lowering=False, debug=False,
                   num_devices=N_CORES)
    sel = nc.dram_tensor("sel", (C, S2), f16, kind="ExternalInput").ap()
    gg = nc.dram_tensor("gg", (C, S2), f16, kind="ExternalInput").ap()
    msk = nc.dram_tensor("msk", (C, S2), u8, kind="ExternalInput").ap()
    ref = nc.dram_tensor("ref", (C, S2), f16, kind="ExternalOutput").ap()

    # stage-I dma/scan chunks: graded so the first scan starts early
    SI = [480, 1152, 3072, 2976]  # multiples of 96 (column-aligned)
    assert sum(SI) == S2
    with tile.TileContext(nc) as tc, ExitStack() as ctx:
        pools = {}
        for name, bufs in [("sel", P2C + 1), ("g", 3), ("m", 3),
                           ("u", 1), ("wpx", 3), ("wexp", 3),
                           ("t", 3), ("rf", 3), ("sm", 1)]:
            pools[name] = ctx.enter_context(tc.tile_pool(name=name, bufs=bufs))
        pT = ctx.enter_context(tc.tile_pool(name="pT", bufs=2, space="PSUM"))
        pW = ctx.enter_context(tc.tile_pool(name="pW", bufs=2, space="PSUM"))
        psm = pools["sm"]

        ident = psm.tile([C, C], f16)
        make_identity(nc, ident[:])
        ident1 = psm.tile([1, 1], f32)
        nc.vector.memset(ident1[:], 1.0)

        mxp = psm.tile([C, len(SI)], f32)
        mcq = psm.tile([1, NS], f32)
        den = psm.tile([1, NS], f32)
        rc1 = psm.tile([1, NS], f32)
        rcr = psm.tile([1, C], f32)

        selts = []
        ubig = pools["u"].tile([C, S2], f16)
        # ---- stage I: scan -> d, per-chunk abs-max ----
        # masks+gaps first (they gate the scans); sel later (needed in II)
        off = 0
        for i, ln in enumerate(SI):
            sl = slice(off, off + ln)
            mt = pools["m"].tile([C, ln], u8, tag="m", bufs=3,
                                 padded_shape=[C, 3072])
            nc.sync.dma_start(mt[:], msk[:, sl])
            gt = pools["g"].tile([C, ln], f16, tag="g", bufs=3,
                                 padded_shape=[C, 3072])
            nc.sync.dma_start(gt[:], gg[:, sl])
            nc.vector.tensor_tensor_scan(ubig[:, sl], mt[:], gt[:], 0.0,
                                         op0=Alu.mult, op1=Alu.add)
            nc.vector.tensor_reduce(
                mxp[:, i:i + 1], ubig[:, sl], mybir.AxisListType.X, Alu.max,
                apply_absolute_value=True)
            off += ln
        for i in range(P2C):
            sl = slice(i * CH2, (i + 1) * CH2)
            selt = pools["sel"].tile([C, CH2], f16, tag="sel")
            nc.sync.dma_start(selt[:], sel[:, sl])
            selts.append(selt)

        # ---- barrier: m_clip per channel -> 1/m_clip as (128,1) column ----
        mxr = psm.tile([C, 1], f16)
        nc.vector.tensor_reduce(mxr[:], mxp[:], mybir.AxisListType.X, Alu.max)
        trow = pT.tile([1, C], f16, tag="wT", space="PSUM")
        nc.tensor.transpose(trow[:], mxr[:], ident[:])
        # max over the 4 w-quarters for each channel; clamp 0 -> tiny so the
        # reciprocal stays finite (a zero max means u==0 everywhere, so any
        # finite scale yields w_px = 0, matching the reference's m==0 -> 1)
        nc.vector.tensor_reduce(
            mcq[:], trow[:].rearrange("o (q c) -> o c q", q=WQ),
            mybir.AxisListType.X, Alu.max)
        nc.vector.tensor_scalar(den[:], mcq[:], CLIP, 1e-30,
                                op0=Alu.mult, op1=Alu.max)
        nc.vector.reciprocal(rc1[:], den[:])
        nc.vector.tensor_copy(
            rcr[:].rearrange("o (q c) -> o q c", q=WQ),
            rc1[:].unsqueeze(1).broadcast_to((1, WQ, NS)))
        rcpp = pW.tile([C, 1], f32, tag="wB", space="PSUM")
        nc.tensor.matmul(rcpp[:], rcr[:], ident1[:], is_transpose=True)

        # ---- stage II: w_px, channel max via PE transposes, blend ----
        wpxs, wBs = [], []
        for i in range(P2C):
            sl = slice(i * CH2, (i + 1) * CH2)
            # w_px = d * (1/m_clip); sign dropped later by the |.| reduce
            wpx = pools["wpx"].tile([C, CH2], f16, tag="wpx", bufs=P2C)
            nc.vector.tensor_scalar(wpx[:], ubig[:, sl], rcpp[:], None,
                                    op0=Alu.mult)
            wpxs.append(wpx)
        for i in range(P2C):
            sl = slice(i * CH2, (i + 1) * CH2)
            wpx = wpxs[i]
            wT = pT.tile([C, CH2], f16, tag="wT", space="PSUM")
            for t in range(NB2):
                ts = slice(t * 128, (t + 1) * 128)
                nc.tensor.transpose(wT[:, ts], wpx[:, ts], ident[:])
            wrd = psm.tile([C, NB2 * WQ], f16, tag=f"wrd{i}")
            nc.vector.tensor_reduce(
                wrd[:], wT[:].rearrange("p (t q c) -> p t q c", q=WQ, c=NS),
                mybir.AxisListType.X, Alu.max, apply_absolute_value=True)
            nc.vector.tensor_scalar(wrd[:], wrd[:], 1.0, None, op0=Alu.min)
            wexp = pools["wexp"].tile([C, CH2], f16, tag="wexp")
            nc.scalar.activation(
                wexp[:].rearrange("p (t q c) -> p t q c", q=WQ, c=NS),
                wrd[:].rearrange("p (t q) -> p t q", q=WQ).unsqueeze(-1)
                .broadcast_to((C, NB2, WQ, NS)),
                Act.Copy)
            wB = pW.tile([C, CH2], f16, tag="wB", space="PSUM")
            for t in range(NB2):
                ts = slice(t * 128, (t + 1) * 128)
                nc.tensor.transpose(wB[:, ts], wexp[:, ts], ident[:])
            wBs.append(wB)
        # t/rf pairwise on DVE so each out-DMA fires as early as possible;
        # chunk 0's rf runs on Pool with its DMA on the ACT queue to avoid
        # head-of-line blocking the SP DMA queue behind the slow Pool op.
        for i in range(P2C):
            sl = slice(i * CH2, (i + 1) * CH2)
            tt = pools["t"].tile([C, CH2], f16, tag="t", bufs=P2C)
            nc.vector.tensor_tensor(
                tt[:], wBs[i][:], ubig[:, sl], Alu.mult)
            rf = pools["rf"].tile([C, CH2], f16, tag="rf", bufs=4)
            if i < 1:
                nc.gpsimd.tensor_tensor(rf[:], tt[:], selts[i][:], Alu.add)
                nc.scalar.dma_start(ref[:, sl], rf[:])
            elif i < P2C - 1:
                nc.vector.tensor_tensor(rf[:], tt[:], selts[i][:], Alu.add)
                nc.sync.dma_start(ref[:, sl], rf[:])
            else:
                hh = CH2 // 2
                nc.vector.tensor_tensor(rf[:, :hh], tt[:, :hh],
                                        selts[i][:, :hh], Alu.add)
                nc.sync.dma_start(ref[:, i * CH2:i * CH2 + hh], rf[:, :hh])
                nc.vector.tensor_tensor(rf[:, hh:], tt[:, hh:],
                                        selts[i][:, hh:], Alu.add)
                nc.sync.dma_start(ref[:, i * CH2 + hh:(i + 1) * CH2],
                                  rf[:, hh:])
    nc.compile()
    return nc


def _disp16():
    return np.linspace(0.1, 1.0, H).astype(np.float32).astype(np.float16)


def _blk_positions():
    """(NBLK, 128) flattened positions: blocks 0..191 hold one image row's
    w in [0,256) (two blocks per row, so a pair shares a constant disp);
    blocks 192..239 hold the w in [256,320) leftovers of two rows each."""
    pos = np.empty((NBLK, 128), np.int64)
    hh = np.arange(H)
    pure = (hh[:, None, None] * W
            + np.arange(256).reshape(2, 128)[None]).reshape(H * 2, 128)
    pos[:192] = pure
    mixed = (np.arange(H).reshape(48, 2)[:, :, None] * W
             + np.arange(256, 320)[None, None, :]).reshape(48, 128)
    pos[192:] = mixed
    return pos


def _pack_phase1(x):
    """x (B,C,H,W) f32 -> xt (B,128,NBLK*128) f16 transposed-block layout,
    vb (128, NBLK*2) f16 stationary [disp, 1] per (pos_in, blk)."""
    pos = _blk_positions()
    xflat = x.transpose(0, 2, 3, 1).reshape(B, HW, C)
    xt = xflat[:, pos.reshape(-1), :].reshape(B, NBLK, 128, C)
    xt = np.ascontiguousarray(xt.transpose(0, 2, 1, 3)).reshape(B, 128, NBLK * C)
    xt = xt.astype(np.float16)

    disp_pos = np.repeat(_disp16(), W)                      # (HW,) f16
    vb = np.empty((NBLK, 128, 2), np.float16)
    vb[:, :, 0] = disp_pos[pos]
    vb[:, :, 1] = 1.0
    vb = np.ascontiguousarray(vb.transpose(1, 0, 2)).reshape(128, NBLK * 2)
    return xt, vb


def _select_channels(acc_sum):
    """acc_sum: f64 (2,512) summed over cores (paired-matmul halves).
    Rank channels by cosine similarity against the disp/depth ramps."""
    half = acc_sum[:, :256] + acc_sum[:, 256:]
    dot_disp = half[0, :C]
    tot = half[1, :C]
    ssq = half[1, C:]
    dot_depth = tot - dot_disp

    d16 = _disp16().astype(np.float64)
    n_rep = B * W
    vn_disp = np.sqrt(n_rep * (d16 @ d16))
    vn_depth = np.sqrt(n_rep * ((1.0 - d16) @ (1.0 - d16)))
    sn = np.maximum(np.sqrt(ssq), EPS)
    cos_disp = dot_disp / (sn * vn_disp)
    cos_depth = dot_depth / (sn * vn_depth)
    disp_idx = np.argsort(-cos_disp, kind="stable")[:NSEL]
    depth_idx = np.argsort(-cos_depth, kind="stable")[:NSEL]
    return np.concatenate([disp_idx, depth_idx])


def _pack_p2_layout(a):
    """(B, NS, Hrev, W) -> (B, 128, S2) with p = wq*32+ch, free = (col, h)."""
    at = a.transpose(0, 1, 3, 2)                            # (B, NS, W, Hrev)
    ap = np.ascontiguousarray(
        at.reshape(B, NS, WQ, WPQ, H).transpose(0, 2, 1, 3, 4)
    ).reshape(B, C, S2)
    return ap


def _pack_phase2(x, dynamic_masks, idx):
    """Pack sel, g = m*(sel_below - sel), and mask into the per-core
    (128, S2) device layout (fp16 / u8)."""
    sel = x[:, idx]                                         # (B, NS, H, W) f32
    sel_r = sel[:, :, ::-1, :]                              # bottom-up
    m_r = (dynamic_masks[:, ::-1, :] != 0)
    m_r = m_r.copy()
    m_r[:, 0, :] = False               # bottom row never pulls
    g_r = np.zeros_like(sel_r)
    g_r[:, :, 1:] = sel_r[:, :, :-1] - sel_r[:, :, 1:]
    g_r *= m_r[:, None].astype(np.float32)

    sel_p = _pack_p2_layout(sel_r).astype(np.float16)
    g_p = _pack_p2_layout(g_r).astype(np.float16)

    m_t = m_r.astype(np.uint8).transpose(0, 2, 1)           # (B, W, Hrev)
    m_q = np.ascontiguousarray(m_t).reshape(B, WQ, S2)
    m_big = np.broadcast_to(m_q[:, :, None, :], (B, WQ, NS, S2))
    m_p = np.ascontiguousarray(m_big).reshape(B, C, S2)
    return sel_p, g_p, m_p


def _unpack_refined(ref_stack):
    """(B, 128, S2) f16 device layout -> (B, NS, H, W) f32."""
    r = ref_stack.astype(np.float32)
    r = r.reshape(B, WQ, NS, WPQ, H).transpose(0, 2, 1, 3, 4)
    r = r.reshape(B, NS, W, H).transpose(0, 1, 3, 2)        # (B, NS, Hrev, W)
    return r[:, :, ::-1, :]


def _get_runners():
    if "run1" not in _cache:
        nc1 = build_phase1()
        _cache["run1"] = _runner(nc1, N_CORES)
        nc2 = build_phase2()
        _cache["run2"] = _runner(nc2, N_CORES)
    return _cache["run1"], _cache["run2"]


def _max_masked_run(dynamic_masks):
    m = (dynamic_masks != 0)
    best = np.zeros((B, W), dtype=np.int32)
    cur = np.zeros((B, W), dtype=np.int32)
    for h in range(H - 1, -1, -1):
        cur = np.where(m[:, h, :], cur + 1, 0)
        best = np.maximum(best, cur)
    return int(best.max())


def kernel(input_features, dynamic_masks):
    input_features = np.asarray(input_features, dtype=np.float32)
    dynamic_masks = np.asarray(dynamic_masks)
    run1, run2 = _get_runners()

    # Phase 1: per-channel similarity statistics on device
    xt, vb = _pack_phase1(input_features)
    in1 = [{"xt": xt[b], "vb": vb} for b in range(B)]
    outs1 = run1(in1)
    acc_sum = np.zeros((2, 512), np.float64)
    for o in outs1:
        acc_sum += o["acc"].astype(np.float64)
    idx = _select_channels(acc_sum)

    # single-scan propagation is exact iff no masked run >= 33
    assert _max_masked_run(dynamic_masks) <= 32, (
        "masked run of >= 33 rows: single-scan shortcut invalid for this input"
    )

    # Phase 2: propagation + blend on device
    sel_p, g_p, m_p = _pack_phase2(input_features, dynamic_masks, idx)
    in2 = [{"sel": sel_p[b], "gg": g_p[b], "msk": m_p[b]} for b in range(B)]
    outs2 = run2(in2)
    ref_stack = np.stack([o["ref"] for o in outs2])
    refined = _unpack_refined(ref_stack)

    out = input_features.copy()
    out[:, idx] = refined
    return out
